# revision 39
# baseline (speedup 1.0000x reference)
"""MFA e-step (mixture of factor analyzers) on 8 Trainium2 NeuronCores.

Math: the reference computes per-component Gaussian log-likelihoods with
covariance C_k = Lam_k Lam_k^T + diag(psi).  Since Q=16 << D=128 we use the
Woodbury identity: with S = diag(psi), M_k = I + Lam_k^T S^-1 Lam_k = T T^T,
U_k = S^-1 Lam_k T^-T:

  maha_k(x) = d^T S^-1 d - ||U_k^T d||^2,   d = x - mu_k

Expanding in x, the per-sample log responsibility becomes

  log_resps[n,k] = z[n,k] - 0.5*r[n]
  z[n,k]  = const_k + x_n . g_k + || (U_k/sqrt2)^T x_n ||^2
  r[n]    = x_n^T S^-1 x_n

r cancels in the normalized output; it only shifts the log-likelihood.
The device computes, per 128-row tile of X (X is fed pre-transposed, D on
partitions):
  P   = X @ Wh          (Wh = [U_k/sqrt2] stacked, [128, 512])  - PE, fp32r
  crs = X @ GC + const  (GC = [g_k], [128, 32]; const via rank-1) - PE
  rr  = (X*X) @ (-0.5/s)                                         - PE
  z   = groupsum_16(P^2) + crs                                   - ACT+DVE
then one batched logsumexp over all 20 tiles (single exp / single ln, so
the ACT engine loads its function tables at most twice).
Host does only the O(K*D*Q) parameter factorization (tiny) and the
shard/unshard.  Sharding: data-parallel over N, 8 ways, no collectives.
"""

import json
import os
import shutil
import tempfile

import numpy as np

import concourse.bacc as bacc
import concourse.bass as bass
import concourse.mybir as mybir
import concourse.tile as tile
from concourse.bass_utils import run_bass_kernel_spmd


def _install_act_tables():
    """Reorder the ACT function-table sets so the one set that covers every
    function this kernel uses (ln, exp, square, identity, copy) comes first.
    walrus assigns each ACTIVATE the first set containing its function, so
    this removes all mid-kernel ACT_TABLE_LOAD switches (~1.3us each)."""
    if os.environ.get("BASS_ACT_ROOT_JSON_PATH"):
        return
    try:
        from neuronxcc.driver.Job import Job
        from neuronxcc.driver.jobs.support.FindActInfo import findActInfoFile

        src = findActInfoFile(Job.getPackageDir(), "gen3")
        d = json.load(open(src))
        sets = d["act_func_sets"]
        best = [s for s in sets if s["name"] == "natural_log_exp_and_others"]
        rest = [s for s in sets if s["name"] != "natural_log_exp_and_others"]
        if not best:
            return
        d["act_func_sets"] = best + rest
        dst_dir = tempfile.mkdtemp(prefix="act_tables_")
        for f in os.listdir(os.path.dirname(src)):
            sp = os.path.join(os.path.dirname(src), f)
            if os.path.isfile(sp) and f != "act_info.json":
                os.symlink(sp, os.path.join(dst_dir, f))
        with open(os.path.join(dst_dir, "act_info.json"), "w") as f:
            json.dump(d, f)
        os.environ["BASS_ACT_ROOT_JSON_PATH"] = os.path.join(
            dst_dir, "act_info.json")
    except Exception:
        pass


if os.environ.get("MFA_ACT_TABLES", "0") == "1":
    _install_act_tables()


def _fast_drain_and_barrier(self, tick_clock, wait_clock):
    """Cheap Tile epilogue: the sync drain already waits on the global
    vector clock (all engines + DMA queues complete), so the two all-engine
    EVSEM butterfly barriers (~5us each) reduce to one semaphore handoff:
    sync -> gpsimd, which then clears the tile semaphores for NEFF re-use."""
    from concourse.vector_clock import ScopedClock as _SC

    nc = self.nc
    drain_inst = nc.sync.drain()
    wait_clock.add_sem_waits(
        drain_inst.ins, _SC({None: tick_clock.global_clock})
    )
    done = nc.alloc_semaphore("tail_done")
    nc.sync.sem_inc(done, 1)
    nc.gpsimd.wait_ge(done, 1)
    popped = nc._tile_sem_poison_stack.pop()
    assert popped is self._sem_poison
    assert self.sems is not None
    nc.clear_and_free_semaphores(list(self.sems.allocated().values()))
    nc.gpsimd.sem_clear(done)
    nc.release_semaphore(done)


if os.environ.get("MFA_FAST_TAIL", "1") == "1":
    tile.TileContext._drain_and_barrier = _fast_drain_and_barrier

K, D, Q, N = 32, 128, 16, 20000
NCORES = 8
NPAD = 20480          # N padded to 8 * 2560
NLOC = NPAD // NCORES  # 2560 rows per core
PT = 128               # rows per tile (partition dim)
NT = NLOC // PT        # 20 tiles per core
KQ = K * Q             # 512

F32 = mybir.dt.float32
F32R = mybir.dt.float32r
F16 = mybir.dt.float16
AX = mybir.AxisListType
ALU = mybir.AluOpType
ACTF = mybir.ActivationFunctionType

USE_F32R = True       # main P matmul in fp32r
AUX_F32R = True       # crs / rr matmuls in fp32r
POOL_REDUCE = False   # grouped sum-of-squares via pool_avg instead of reduce
SQ_F16 = False        # P^2 stored as fp16 (no DVE speedup observed; off)
SOFT_LN = True        # ln(ssum) in software on GpSimd; ACT keeps one table
PH2_CHUNKS = 2        # logsumexp phases overlapping the main loop
XCHUNKS = 5           # X DMA + fp32r cast pipelined in this many chunks
PSP_BUFS = 3
SQ_BUFS = 4


def build_bass():
    """Build the per-core Tile program (same NEFF on all 8 cores)."""
    nc = bacc.Bacc("TRN2", target_bir_lowering=False, debug=False)

    # X shard arrives pre-transposed: [D, NLOC], so tiles DMA straight into
    # the matmul operand layout (D on partitions) with no on-chip transpose.
    XsT = nc.dram_tensor("XsT", [D, NLOC], F32, kind="ExternalInput")
    Wh = nc.dram_tensor("Wh", [D, KQ], F32, kind="ExternalInput")
    GC = nc.dram_tensor("GC", [D, K], F32, kind="ExternalInput")
    sneg = nc.dram_tensor("sneg", [D, 2], F32, kind="ExternalInput")
    constb = nc.dram_tensor("constb", [PT, NT, K], F32, kind="ExternalInput")
    out_norm = nc.dram_tensor("out_norm", [NLOC, K], F32, kind="ExternalOutput")
    out_ll = nc.dram_tensor("out_ll", [NLOC, 1], F32, kind="ExternalOutput")

    with tile.TileContext(nc) as tc:
        with (
            tc.tile_pool(name="consts", bufs=1) as cpool,
            tc.tile_pool(name="xbig", bufs=1) as xbig,
            tc.tile_pool(name="sq", bufs=SQ_BUFS) as sq_pool,
            tc.tile_pool(name="acc", bufs=1) as accp,
            tc.tile_pool(name="small", bufs=2) as spool,
            tc.tile_pool(name="pP", bufs=PSP_BUFS,
                         space=bass.MemorySpace.PSUM) as psP_pool,
            tc.tile_pool(name="pC", bufs=3, space=bass.MemorySpace.PSUM) as psC_pool,
            tc.tile_pool(name="pR", bufs=2, space=bass.MemorySpace.PSUM) as psR_pool,
        ):
            def load_const(name, dram, shape, rdt):
                # consts ride the Activation HWDGE queue so the X-shard DMAs
                # own the sync queue
                t = cpool.tile(shape, F32, tag=name)
                nc.scalar.dma_start(out=t[:], in_=dram[:])
                if rdt == F32:
                    return t
                tr = cpool.tile(shape, F32R, tag=name + "_r")
                nc.vector.tensor_copy(tr[:], t[:])
                return tr

            auxdt = F32R if AUX_F32R else F32
            maindt = F32R if USE_F32R else F32
            # wh first: it gates the first matmul
            wh_t = load_const("wh", Wh, [D, KQ], maindt)
            gc_t = load_const("gc", GC, [D, K], auxdt)
            # fp32r matmuls need an even output free size; sneg is [D,2]
            # host-side with a zero second column.
            sneg_t = load_const("sneg", sneg, [D, 2], auxdt)
            constb_t = cpool.tile([PT, NT, K], F32, tag="constb")
            nc.scalar.dma_start(out=constb_t[:], in_=constb[:])

            # whole X shard in SBUF, transposed layout [D, NLOC]; DMA, fp32r
            # cast, and x^2 all pipelined in XCHUNKS chunks so the first
            # matmul can start early.
            xt_all = xbig.tile([D, NLOC], F32)
            if USE_F32R or AUX_F32R:
                xtr_all = xbig.tile([D, NLOC], F32R)
            else:
                xtr_all = xt_all
            x2_all = xbig.tile([D, NLOC], F32R if AUX_F32R else F32)
            # small first chunk so the first matmul can start early; the
            # rest in larger chunks, all on the sync queue
            bounds = [0, 2 * PT]
            rem = NLOC - 2 * PT
            nchunk = XCHUNKS - 1
            step = rem // nchunk
            for c in range(nchunk):
                bounds.append(bounds[-1] + step)
            bounds[-1] = NLOC
            for c in range(len(bounds) - 1):
                cs = slice(bounds[c], bounds[c + 1])
                nc.sync.dma_start(out=xt_all[:, cs], in_=XsT[:, cs])
                if USE_F32R or AUX_F32R:
                    nc.vector.tensor_copy(xtr_all[:, cs], xt_all[:, cs])
                nc.scalar.square(x2_all[:, cs], xt_all[:, cs])

            xm_all = xtr_all if USE_F32R else xt_all
            xa_all = xtr_all if AUX_F32R else xt_all

            # accumulators across all tiles
            z_all = accp.tile([PT, NT, K], F32)     # z per (row, tile, k)
            rr_all = accp.tile([PT, NT], F32)       # -0.5 r per (row, tile)
            ev = accp.tile([PT, NT, K], F32)
            outn = accp.tile([PT, NT, K], F32)

            CH = NT // PH2_CHUNKS

            def phase2(c):
                """Batched logsumexp for tiles [c*CH, (c+1)*CH)."""
                ts = slice(c * CH, (c + 1) * CH)
                # fold in the per-component constant, one GpSimd op per chunk
                nc.gpsimd.tensor_add(z_all[:, ts, :], z_all[:, ts, :],
                                     constb_t[:, ts, :])
                zf = z_all[:, ts, :].rearrange("p t k -> p (t k)")
                negm = spool.tile([PT, 1], F32, tag="negm")
                nc.vector.tensor_reduce(negm[:], zf, axis=AX.X, op=ALU.max,
                                        negate=True)
                nc.scalar.activation(
                    ev[:, ts, :].rearrange("p t k -> p (t k)"), zf,
                    ACTF.Exp, bias=negm[:, 0:1], scale=1.0)
                ssum = spool.tile([PT, CH], F32, tag="ssum")
                nc.vector.tensor_reduce(ssum[:], ev[:, ts, :], axis=AX.X,
                                        op=ALU.add)
                lg = spool.tile([PT, CH], F32, tag="lg")
                if SOFT_LN:
                    # ln on GpSimd (keeps the ACT engine on a single table
                    # set): ln(y) = (e-127)ln2 + 2*atanh(s), s=(m-1)/(m+1),
                    # atanh(s) ~ s*(1 + s^2/3 + s^4/5), |s|<0.1716.
                    I32 = mybir.dt.int32
                    bits = ssum[:].bitcast(I32)
                    e_i = spool.tile([PT, CH], I32, tag="ln_ei")
                    nc.vector.tensor_scalar(e_i[:], bits, 23, None,
                                            op0=ALU.arith_shift_right)
                    e_f = spool.tile([PT, CH], F32, tag="ln_ef")
                    nc.vector.tensor_copy(e_f[:], e_i[:])   # int -> float
                    LN2 = 0.6931471805599453
                    et = spool.tile([PT, CH], F32, tag="ln_et")
                    nc.vector.tensor_scalar(et[:], e_f[:], LN2, -127.0 * LN2,
                                            op0=ALU.mult, op1=ALU.add)
                    mb = spool.tile([PT, CH], I32, tag="ln_mb")
                    nc.vector.tensor_scalar(mb[:], bits, 0x007FFFFF,
                                            0x3F800000,
                                            op0=ALU.bitwise_and,
                                            op1=ALU.bitwise_or)
                    mant = mb[:].bitcast(F32)
                    num = spool.tile([PT, CH], F32, tag="ln_num")
                    nc.vector.tensor_scalar(num[:], mant, 1.0, None,
                                            op0=ALU.subtract)
                    den = spool.tile([PT, CH], F32, tag="ln_den")
                    nc.vector.tensor_scalar(den[:], mant, 1.0, None,
                                            op0=ALU.add)
                    rden = spool.tile([PT, CH], F32, tag="ln_rden")
                    nc.vector.reciprocal(rden[:], den[:])
                    sv = spool.tile([PT, CH], F32, tag="ln_s")
                    nc.vector.tensor_tensor(sv[:], num[:], rden[:],
                                            op=ALU.mult)
                    s2 = spool.tile([PT, CH], F32, tag="ln_s2")
                    nc.vector.tensor_tensor(s2[:], sv[:], sv[:],
                                            op=ALU.mult)
                    tpoly = spool.tile([PT, CH], F32, tag="ln_t")
                    nc.vector.tensor_scalar(tpoly[:], s2[:], 0.2, None,
                                            op0=ALU.mult)
                    nc.vector.scalar_tensor_tensor(
                        tpoly[:], tpoly[:], 1.0 / 3.0, s2[:],
                        op0=ALU.add, op1=ALU.mult)
                    lnm2 = spool.tile([PT, CH], F32, tag="ln_lnm2")
                    nc.vector.scalar_tensor_tensor(
                        lnm2[:], tpoly[:], 1.0, sv[:],
                        op0=ALU.add, op1=ALU.mult)
                    nc.vector.scalar_tensor_tensor(
                        lg[:], lnm2[:], 2.0, et[:],
                        op0=ALU.mult, op1=ALU.add)
                else:
                    nc.scalar.activation(lg[:], ssum[:], ACTF.Ln)
                # lse[p,t] = lg[p,t] + m[p] = lg - negm
                lse = spool.tile([PT, CH], F32, tag="lse")
                nc.vector.tensor_scalar(lse[:], lg[:], negm[:, 0:1], None,
                                        op0=ALU.subtract)
                # ll = lse + rr
                ll = spool.tile([PT, CH], F32, tag="ll")
                nc.vector.tensor_add(ll[:], lse[:], rr_all[:, ts])
                # outn = z - lse (broadcast along k)
                lse_b = lse[:].unsqueeze(2).broadcast_to([PT, CH, K])
                nc.vector.tensor_sub(outn[:, ts, :], z_all[:, ts, :], lse_b)

                # out_norm[(t*128+p), k] = outn[p, t, k]
                on_view = out_norm.ap().rearrange("(t p) k -> p t k", p=PT)
                nc.sync.dma_start(out=on_view[:, ts, :], in_=outn[:, ts, :])
                oll_view = out_ll.ap().rearrange("(t p) one -> p (t one)",
                                                 p=PT)
                nc.sync.dma_start(out=oll_view[:, ts], in_=ll[:])

            for i in range(NT):
                cols = slice(i * PT, (i + 1) * PT)

                # P = X @ Wh   -> [n, 512]
                psP = psP_pool.tile([PT, KQ], F32, tag="psP")
                nc.tensor.matmul(psP[:], xm_all[:, cols], wh_t[:],
                                 start=True, stop=True)
                # crs = X @ GC -> [n, 32]
                psC = psC_pool.tile([PT, K], F32, tag="psC")
                nc.tensor.matmul(psC[:], xa_all[:, cols], gc_t[:],
                                 start=True, stop=True)
                # rr = (X*X) @ sneg -> [n, 1]
                psR = psR_pool.tile([PT, 2], F32, tag="psR")
                nc.tensor.matmul(psR[:], x2_all[:, cols], sneg_t[:],
                                 start=True, stop=True)

                # sq = P^2 (ACT, PSUM->SBUF)
                sq = sq_pool.tile([PT, KQ], F16 if SQ_F16 else F32, tag="sq")
                nc.scalar.square(sq[:], psP[:])

                # rr slice first (on ACT): frees psR early
                nc.scalar.copy(rr_all[:, i:i + 1], psR[:, 0:1])

                # z0[n,k] = sum_q sq[n, k*16+q] (DVE grouped reduce; fp16
                # in+out lets the DVE run its 2x mode)
                sqg = sq[:].rearrange("p (k q) -> p k q", q=Q)
                z0 = spool.tile([PT, K], F16 if SQ_F16 else F32, tag="z0")
                with nc.allow_low_precision("z0 ~ O(30), fp16 err ~2e-2"):
                    nc.vector.tensor_reduce(z0[:], sqg, axis=AX.X, op=ALU.add)
                # z = z0 + crs (DVE reads PSUM; frees psC); the
                # per-component constant is folded in once per phase2 chunk
                nc.vector.tensor_add(z_all[:, i, :], z0[:], psC[:])

                if (i + 1) % CH == 0:
                    phase2(i // CH)

    nc.compile()
    return nc


def host_precompute(X, log_pi, mu, Lam, log_psi):
    """Tiny O(K*D*Q) parameter factorization, in float64 for accuracy."""
    log_pi = np.asarray(log_pi, np.float64)
    mu = np.asarray(mu, np.float64)
    Lam = np.asarray(Lam, np.float64)
    log_psi = np.asarray(log_psi, np.float64)

    s = np.exp(log_psi) + 1e-5 + 1e-4                       # [D]
    sinv = 1.0 / s
    B = Lam * (s ** -0.5)[None, :, None]                    # [K,D,Q]
    M = np.eye(Q)[None] + np.einsum('kdq,kdr->kqr', B, B)   # [K,Q,Q]
    T = np.linalg.cholesky(M)
    logdet = np.sum(np.log(s)) + 2.0 * np.log(
        np.diagonal(T, axis1=1, axis2=2)).sum(1)            # [K]
    Tinv = np.linalg.inv(T)
    U = np.einsum('d,kdq,krq->kdr', sinv, Lam, Tinv)        # [K,D,Q]
    a = sinv[None, :] * mu                                  # [K,D]
    c = np.einsum('kdq,kd->kq', U, mu)                      # [K,Q]
    v = np.einsum('kdq,kq->kd', U, c)                       # [K,D]
    g = a - v                                               # [K,D]
    q1 = np.einsum('kd,kd->k', mu, a)
    q2 = np.einsum('kq,kq->k', c, c)
    const = (log_pi - 0.5 * (D * np.log(2 * np.pi) + logdet)
             - 0.5 * q1 + 0.5 * q2)                         # [K]

    # scale so that the device's grouped reduce (plain sum, or avg-pool which
    # divides by Q) yields exactly 0.5 * ||U^T x||^2
    wscale = np.sqrt(Q / 2.0) if POOL_REDUCE else np.sqrt(0.5)
    Wh = (U * wscale).transpose(0, 2, 1).reshape(KQ, D).T  # [D, KQ]
    return {
        "Wh": np.ascontiguousarray(Wh, dtype=np.float32),
        "GC": np.ascontiguousarray(g.T, dtype=np.float32),
        "sneg": np.ascontiguousarray(
            np.stack([-0.5 * sinv, np.zeros(D)], axis=1), dtype=np.float32),
        "constb": np.ascontiguousarray(
            np.broadcast_to(const[None, None, :], (PT, NT, K)),
            dtype=np.float32),
    }


_NC_CACHE = None


def get_nc():
    global _NC_CACHE
    if _NC_CACHE is None:
        _NC_CACHE = build_bass()
    return _NC_CACHE


def kernel(X, log_pi, mu, Lam, log_psi, _collect=None):
    X = np.asarray(X, np.float32)
    params = host_precompute(X, log_pi, mu, Lam, log_psi)

    Xpad = np.zeros((NPAD, D), dtype=np.float32)
    Xpad[:N] = X
    # per-core transposed shards [D, NLOC]
    shards = Xpad.reshape(NCORES, NLOC, D)

    in_maps = [dict(params, XsT=np.ascontiguousarray(shards[c].T))
               for c in range(NCORES)]

    nc = get_nc()
    res = run_bass_kernel_spmd(nc, in_maps, list(range(NCORES)),
                               **(_collect or {}))
    if _collect is not None:
        _collect["res"] = res

    # device row order within a core is (t*128 + p); it matches the shard's
    # natural row order, so plain concatenation restores global order.
    norm = np.concatenate([res.results[c]["out_norm"] for c in range(NCORES)],
                          axis=0)[:N]
    ll = np.concatenate([res.results[c]["out_ll"] for c in range(NCORES)],
                        axis=0)[:N, 0]
    return norm, ll


# revision 40
# speedup vs baseline: 1.1022x; 1.1022x over previous
"""MFA e-step (mixture of factor analyzers) on 8 Trainium2 NeuronCores.

Math: the reference computes per-component Gaussian log-likelihoods with
covariance C_k = Lam_k Lam_k^T + diag(psi).  Since Q=16 << D=128 we use the
Woodbury identity: with S = diag(psi), M_k = I + Lam_k^T S^-1 Lam_k = T T^T,
U_k = S^-1 Lam_k T^-T:

  maha_k(x) = d^T S^-1 d - ||U_k^T d||^2,   d = x - mu_k

Expanding in x, the per-sample log responsibility becomes

  log_resps[n,k] = z[n,k] - 0.5*r[n]
  z[n,k]  = const_k + x_n . g_k + || (U_k/sqrt2)^T x_n ||^2
  r[n]    = x_n^T S^-1 x_n

r cancels in the normalized output; it only shifts the log-likelihood.
The device computes, per 128-row tile of X (X is fed pre-transposed, D on
partitions):
  P   = X @ Wh          (Wh = [U_k/sqrt2] stacked, [128, 512])  - PE, fp32r
  crs = X @ GC + const  (GC = [g_k], [128, 32]; const via rank-1) - PE
  rr  = (X*X) @ (-0.5/s)                                         - PE
  z   = groupsum_16(P^2) + crs                                   - ACT+DVE
then one batched logsumexp over all 20 tiles (single exp / single ln, so
the ACT engine loads its function tables at most twice).
Host does only the O(K*D*Q) parameter factorization (tiny) and the
shard/unshard.  Sharding: data-parallel over N, 8 ways, no collectives.
"""

import json
import os
import shutil
import tempfile

import numpy as np

import concourse.bacc as bacc
import concourse.bass as bass
import concourse.mybir as mybir
import concourse.tile as tile
from concourse.bass_utils import run_bass_kernel_spmd


def _install_act_tables():
    """Reorder the ACT function-table sets so the one set that covers every
    function this kernel uses (ln, exp, square, identity, copy) comes first.
    walrus assigns each ACTIVATE the first set containing its function, so
    this removes all mid-kernel ACT_TABLE_LOAD switches (~1.3us each)."""
    if os.environ.get("BASS_ACT_ROOT_JSON_PATH"):
        return
    try:
        from neuronxcc.driver.Job import Job
        from neuronxcc.driver.jobs.support.FindActInfo import findActInfoFile

        src = findActInfoFile(Job.getPackageDir(), "gen3")
        d = json.load(open(src))
        sets = d["act_func_sets"]
        best = [s for s in sets if s["name"] == "natural_log_exp_and_others"]
        rest = [s for s in sets if s["name"] != "natural_log_exp_and_others"]
        if not best:
            return
        d["act_func_sets"] = best + rest
        dst_dir = tempfile.mkdtemp(prefix="act_tables_")
        for f in os.listdir(os.path.dirname(src)):
            sp = os.path.join(os.path.dirname(src), f)
            if os.path.isfile(sp) and f != "act_info.json":
                os.symlink(sp, os.path.join(dst_dir, f))
        with open(os.path.join(dst_dir, "act_info.json"), "w") as f:
            json.dump(d, f)
        os.environ["BASS_ACT_ROOT_JSON_PATH"] = os.path.join(
            dst_dir, "act_info.json")
    except Exception:
        pass


if os.environ.get("MFA_ACT_TABLES", "0") == "1":
    _install_act_tables()


def _fast_drain_and_barrier(self, tick_clock, wait_clock):
    """Cheap Tile epilogue: the sync drain already waits on the global
    vector clock (all engines + DMA queues complete), so the two all-engine
    EVSEM butterfly barriers (~5us each) reduce to one semaphore handoff:
    sync -> gpsimd, which then clears the tile semaphores for NEFF re-use."""
    from concourse.vector_clock import ScopedClock as _SC

    nc = self.nc
    drain_inst = nc.sync.drain()
    wait_clock.add_sem_waits(
        drain_inst.ins, _SC({None: tick_clock.global_clock})
    )
    done = nc.alloc_semaphore("tail_done")
    nc.sync.sem_inc(done, 1)
    nc.gpsimd.wait_ge(done, 1)
    popped = nc._tile_sem_poison_stack.pop()
    assert popped is self._sem_poison
    assert self.sems is not None
    nc.clear_and_free_semaphores(list(self.sems.allocated().values()))
    nc.gpsimd.sem_clear(done)
    nc.release_semaphore(done)


if os.environ.get("MFA_FAST_TAIL", "1") == "1":
    tile.TileContext._drain_and_barrier = _fast_drain_and_barrier

K, D, Q, N = 32, 128, 16, 20000
NCORES = 8
NPAD = 20480          # N padded to 8 * 2560
NLOC = NPAD // NCORES  # 2560 rows per core
PT = 128               # rows per tile (partition dim)
NT = NLOC // PT        # 20 tiles per core
KQ = K * Q             # 512

F32 = mybir.dt.float32
F32R = mybir.dt.float32r
F16 = mybir.dt.float16
AX = mybir.AxisListType
ALU = mybir.AluOpType
ACTF = mybir.ActivationFunctionType

USE_F32R = True       # main P matmul in fp32r
AUX_F32R = True       # crs / rr matmuls in fp32r
POOL_REDUCE = False   # grouped sum-of-squares via pool_avg instead of reduce
SQ_F16 = False        # P^2 stored as fp16 (no DVE speedup observed; off)
SOFT_LN = True        # ln(ssum) in software on GpSimd; ACT keeps one table
PH2_CHUNKS = 2        # logsumexp phases overlapping the main loop
XCHUNKS = 5           # X DMA + fp32r cast pipelined in this many chunks
PSP_BUFS = 3
SQ_BUFS = 4


def build_bass():
    """Build the per-core Tile program (same NEFF on all 8 cores)."""
    nc = bacc.Bacc("TRN2", target_bir_lowering=False, debug=False)

    # X shard arrives pre-transposed: [D, NLOC], so tiles DMA straight into
    # the matmul operand layout (D on partitions) with no on-chip transpose.
    XsT = nc.dram_tensor("XsT", [D, NLOC], F32, kind="ExternalInput")
    Wh = nc.dram_tensor("Wh", [D, KQ], F32, kind="ExternalInput")
    GC = nc.dram_tensor("GC", [D, K], F32, kind="ExternalInput")
    sneg = nc.dram_tensor("sneg", [D, 2], F32, kind="ExternalInput")
    constb = nc.dram_tensor("constb", [PT, K], F32, kind="ExternalInput")
    out_norm = nc.dram_tensor("out_norm", [NLOC, K], F32, kind="ExternalOutput")
    out_ll = nc.dram_tensor("out_ll", [NLOC, 1], F32, kind="ExternalOutput")

    with tile.TileContext(nc) as tc:
        with (
            tc.tile_pool(name="consts", bufs=1) as cpool,
            tc.tile_pool(name="xbig", bufs=1) as xbig,
            tc.tile_pool(name="sq", bufs=SQ_BUFS) as sq_pool,
            tc.tile_pool(name="acc", bufs=1) as accp,
            tc.tile_pool(name="small", bufs=2) as spool,
            tc.tile_pool(name="pP", bufs=PSP_BUFS,
                         space=bass.MemorySpace.PSUM) as psP_pool,
            tc.tile_pool(name="pC", bufs=3, space=bass.MemorySpace.PSUM) as psC_pool,
            tc.tile_pool(name="pR", bufs=2, space=bass.MemorySpace.PSUM) as psR_pool,
        ):
            def load_const(name, dram, shape, rdt):
                t = cpool.tile(shape, F32, tag=name)
                nc.sync.dma_start(out=t[:], in_=dram[:])
                if rdt == F32:
                    return t
                tr = cpool.tile(shape, F32R, tag=name + "_r")
                nc.vector.tensor_copy(tr[:], t[:])
                return tr

            auxdt = F32R if AUX_F32R else F32
            maindt = F32R if USE_F32R else F32
            # wh first: it gates the first matmul
            wh_t = load_const("wh", Wh, [D, KQ], maindt)
            gc_t = load_const("gc", GC, [D, K], auxdt)
            # fp32r matmuls need an even output free size; sneg is [D,2]
            # host-side with a zero second column.
            sneg_t = load_const("sneg", sneg, [D, 2], auxdt)
            constb_t = cpool.tile([PT, K], F32, tag="constb")
            nc.sync.dma_start(out=constb_t[:], in_=constb[:])

            # whole X shard in SBUF, transposed layout [D, NLOC]; DMA, fp32r
            # cast, and x^2 all pipelined in XCHUNKS chunks so the first
            # matmul can start early.
            xt_all = xbig.tile([D, NLOC], F32)
            if USE_F32R or AUX_F32R:
                xtr_all = xbig.tile([D, NLOC], F32R)
            else:
                xtr_all = xt_all
            x2_all = xbig.tile([D, NLOC], F32R if AUX_F32R else F32)
            XC = NLOC // XCHUNKS
            for c in range(XCHUNKS):
                cs = slice(c * XC, (c + 1) * XC)
                nc.sync.dma_start(out=xt_all[:, cs], in_=XsT[:, cs])
                if USE_F32R or AUX_F32R:
                    nc.vector.tensor_copy(xtr_all[:, cs], xt_all[:, cs])
                nc.scalar.square(x2_all[:, cs], xt_all[:, cs])

            xm_all = xtr_all if USE_F32R else xt_all
            xa_all = xtr_all if AUX_F32R else xt_all

            # accumulators across all tiles
            z_all = accp.tile([PT, NT, K], F32)     # z per (row, tile, k)
            rr_all = accp.tile([PT, NT], F32)       # -0.5 r per (row, tile)
            ev = accp.tile([PT, NT, K], F32)
            outn = accp.tile([PT, NT, K], F32)

            CH = NT // PH2_CHUNKS

            def phase2(c):
                """Batched logsumexp for tiles [c*CH, (c+1)*CH)."""
                ts = slice(c * CH, (c + 1) * CH)
                zf = z_all[:, ts, :].rearrange("p t k -> p (t k)")
                negm = spool.tile([PT, 1], F32, tag="negm")
                nc.vector.tensor_reduce(negm[:], zf, axis=AX.X, op=ALU.max,
                                        negate=True)
                nc.scalar.activation(
                    ev[:, ts, :].rearrange("p t k -> p (t k)"), zf,
                    ACTF.Exp, bias=negm[:, 0:1], scale=1.0)
                ssum = spool.tile([PT, CH], F32, tag="ssum")
                nc.vector.tensor_reduce(ssum[:], ev[:, ts, :], axis=AX.X,
                                        op=ALU.add)
                lg = spool.tile([PT, CH], F32, tag="lg")
                if SOFT_LN:
                    # ln on GpSimd (keeps the ACT engine on a single table
                    # set): ln(y) = (e-127)ln2 + 2*atanh(s), s=(m-1)/(m+1),
                    # atanh(s) ~ s*(1 + s^2/3 + s^4/5), |s|<0.1716.
                    I32 = mybir.dt.int32
                    bits = ssum[:].bitcast(I32)
                    e_i = spool.tile([PT, CH], I32, tag="ln_ei")
                    nc.vector.tensor_scalar(e_i[:], bits, 23, None,
                                            op0=ALU.arith_shift_right)
                    e_f = spool.tile([PT, CH], F32, tag="ln_ef")
                    nc.vector.tensor_copy(e_f[:], e_i[:])   # int -> float
                    LN2 = 0.6931471805599453
                    et = spool.tile([PT, CH], F32, tag="ln_et")
                    nc.vector.tensor_scalar(et[:], e_f[:], LN2, -127.0 * LN2,
                                            op0=ALU.mult, op1=ALU.add)
                    mb = spool.tile([PT, CH], I32, tag="ln_mb")
                    nc.vector.tensor_scalar(mb[:], bits, 0x007FFFFF,
                                            0x3F800000,
                                            op0=ALU.bitwise_and,
                                            op1=ALU.bitwise_or)
                    mant = mb[:].bitcast(F32)
                    num = spool.tile([PT, CH], F32, tag="ln_num")
                    nc.vector.tensor_scalar(num[:], mant, 1.0, None,
                                            op0=ALU.subtract)
                    den = spool.tile([PT, CH], F32, tag="ln_den")
                    nc.vector.tensor_scalar(den[:], mant, 1.0, None,
                                            op0=ALU.add)
                    rden = spool.tile([PT, CH], F32, tag="ln_rden")
                    nc.vector.reciprocal(rden[:], den[:])
                    sv = spool.tile([PT, CH], F32, tag="ln_s")
                    nc.vector.tensor_tensor(sv[:], num[:], rden[:],
                                            op=ALU.mult)
                    s2 = spool.tile([PT, CH], F32, tag="ln_s2")
                    nc.vector.tensor_tensor(s2[:], sv[:], sv[:],
                                            op=ALU.mult)
                    tpoly = spool.tile([PT, CH], F32, tag="ln_t")
                    nc.vector.tensor_scalar(tpoly[:], s2[:], 0.2, None,
                                            op0=ALU.mult)
                    nc.vector.scalar_tensor_tensor(
                        tpoly[:], tpoly[:], 1.0 / 3.0, s2[:],
                        op0=ALU.add, op1=ALU.mult)
                    lnm2 = spool.tile([PT, CH], F32, tag="ln_lnm2")
                    nc.vector.scalar_tensor_tensor(
                        lnm2[:], tpoly[:], 1.0, sv[:],
                        op0=ALU.add, op1=ALU.mult)
                    nc.vector.scalar_tensor_tensor(
                        lg[:], lnm2[:], 2.0, et[:],
                        op0=ALU.mult, op1=ALU.add)
                else:
                    nc.scalar.activation(lg[:], ssum[:], ACTF.Ln)
                # lse[p,t] = lg[p,t] + m[p] = lg - negm
                lse = spool.tile([PT, CH], F32, tag="lse")
                nc.vector.tensor_scalar(lse[:], lg[:], negm[:, 0:1], None,
                                        op0=ALU.subtract)
                # ll = lse + rr
                ll = spool.tile([PT, CH], F32, tag="ll")
                nc.vector.tensor_add(ll[:], lse[:], rr_all[:, ts])
                # outn = z - lse (broadcast along k)
                lse_b = lse[:].unsqueeze(2).broadcast_to([PT, CH, K])
                nc.vector.tensor_sub(outn[:, ts, :], z_all[:, ts, :], lse_b)

                # out_norm[(t*128+p), k] = outn[p, t, k]
                on_view = out_norm.ap().rearrange("(t p) k -> p t k", p=PT)
                nc.sync.dma_start(out=on_view[:, ts, :], in_=outn[:, ts, :])
                oll_view = out_ll.ap().rearrange("(t p) one -> p (t one)",
                                                 p=PT)
                nc.sync.dma_start(out=oll_view[:, ts], in_=ll[:])

            for i in range(NT):
                cols = slice(i * PT, (i + 1) * PT)

                # P = X @ Wh   -> [n, 512]
                psP = psP_pool.tile([PT, KQ], F32, tag="psP")
                nc.tensor.matmul(psP[:], xm_all[:, cols], wh_t[:],
                                 start=True, stop=True)
                # crs = X @ GC -> [n, 32]
                psC = psC_pool.tile([PT, K], F32, tag="psC")
                nc.tensor.matmul(psC[:], xa_all[:, cols], gc_t[:],
                                 start=True, stop=True)
                # rr = (X*X) @ sneg -> [n, 1]
                psR = psR_pool.tile([PT, 2], F32, tag="psR")
                nc.tensor.matmul(psR[:], x2_all[:, cols], sneg_t[:],
                                 start=True, stop=True)

                # sq = P^2 (ACT, PSUM->SBUF)
                sq = sq_pool.tile([PT, KQ], F16 if SQ_F16 else F32, tag="sq")
                nc.scalar.square(sq[:], psP[:])

                # rr slice first: frees psR early
                nc.vector.tensor_copy(rr_all[:, i:i + 1], psR[:, 0:1])

                # z0[n,k] = sum_q sq[n, k*16+q] (DVE grouped reduce; fp16
                # in+out lets the DVE run its 2x mode)
                sqg = sq[:].rearrange("p (k q) -> p k q", q=Q)
                z0 = spool.tile([PT, K], F16 if SQ_F16 else F32, tag="z0")
                with nc.allow_low_precision("z0 ~ O(30), fp16 err ~2e-2"):
                    nc.vector.tensor_reduce(z0[:], sqg, axis=AX.X, op=ALU.add)
                # z = z0 + crs (DVE reads PSUM; frees psC), then += const
                nc.vector.tensor_add(z_all[:, i, :], z0[:], psC[:])
                nc.gpsimd.tensor_add(z_all[:, i, :], z_all[:, i, :],
                                     constb_t[:])

                if (i + 1) % CH == 0:
                    phase2(i // CH)

    nc.compile()
    return nc


def host_precompute(X, log_pi, mu, Lam, log_psi):
    """Tiny O(K*D*Q) parameter factorization, in float64 for accuracy."""
    log_pi = np.asarray(log_pi, np.float64)
    mu = np.asarray(mu, np.float64)
    Lam = np.asarray(Lam, np.float64)
    log_psi = np.asarray(log_psi, np.float64)

    s = np.exp(log_psi) + 1e-5 + 1e-4                       # [D]
    sinv = 1.0 / s
    B = Lam * (s ** -0.5)[None, :, None]                    # [K,D,Q]
    M = np.eye(Q)[None] + np.einsum('kdq,kdr->kqr', B, B)   # [K,Q,Q]
    T = np.linalg.cholesky(M)
    logdet = np.sum(np.log(s)) + 2.0 * np.log(
        np.diagonal(T, axis1=1, axis2=2)).sum(1)            # [K]
    Tinv = np.linalg.inv(T)
    U = np.einsum('d,kdq,krq->kdr', sinv, Lam, Tinv)        # [K,D,Q]
    a = sinv[None, :] * mu                                  # [K,D]
    c = np.einsum('kdq,kd->kq', U, mu)                      # [K,Q]
    v = np.einsum('kdq,kq->kd', U, c)                       # [K,D]
    g = a - v                                               # [K,D]
    q1 = np.einsum('kd,kd->k', mu, a)
    q2 = np.einsum('kq,kq->k', c, c)
    const = (log_pi - 0.5 * (D * np.log(2 * np.pi) + logdet)
             - 0.5 * q1 + 0.5 * q2)                         # [K]

    # scale so that the device's grouped reduce (plain sum, or avg-pool which
    # divides by Q) yields exactly 0.5 * ||U^T x||^2
    wscale = np.sqrt(Q / 2.0) if POOL_REDUCE else np.sqrt(0.5)
    Wh = (U * wscale).transpose(0, 2, 1).reshape(KQ, D).T  # [D, KQ]
    return {
        "Wh": np.ascontiguousarray(Wh, dtype=np.float32),
        "GC": np.ascontiguousarray(g.T, dtype=np.float32),
        "sneg": np.ascontiguousarray(
            np.stack([-0.5 * sinv, np.zeros(D)], axis=1), dtype=np.float32),
        "constb": np.ascontiguousarray(
            np.broadcast_to(const[None, :], (PT, K)), dtype=np.float32),
    }


_NC_CACHE = None


def get_nc():
    global _NC_CACHE
    if _NC_CACHE is None:
        _NC_CACHE = build_bass()
    return _NC_CACHE


def kernel(X, log_pi, mu, Lam, log_psi, _collect=None):
    X = np.asarray(X, np.float32)
    params = host_precompute(X, log_pi, mu, Lam, log_psi)

    Xpad = np.zeros((NPAD, D), dtype=np.float32)
    Xpad[:N] = X
    # per-core transposed shards [D, NLOC]
    shards = Xpad.reshape(NCORES, NLOC, D)

    in_maps = [dict(params, XsT=np.ascontiguousarray(shards[c].T))
               for c in range(NCORES)]

    nc = get_nc()
    res = run_bass_kernel_spmd(nc, in_maps, list(range(NCORES)),
                               **(_collect or {}))
    if _collect is not None:
        _collect["res"] = res

    # device row order within a core is (t*128 + p); it matches the shard's
    # natural row order, so plain concatenation restores global order.
    norm = np.concatenate([res.results[c]["out_norm"] for c in range(NCORES)],
                          axis=0)[:N]
    ll = np.concatenate([res.results[c]["out_ll"] for c in range(NCORES)],
                        axis=0)[:N, 0]
    return norm, ll


# revision 41
# speedup vs baseline: 1.1757x; 1.0667x over previous
"""MFA e-step (mixture of factor analyzers) on 8 Trainium2 NeuronCores.

Math: the reference computes per-component Gaussian log-likelihoods with
covariance C_k = Lam_k Lam_k^T + diag(psi).  Since Q=16 << D=128 we use the
Woodbury identity: with S = diag(psi), M_k = I + Lam_k^T S^-1 Lam_k = T T^T,
U_k = S^-1 Lam_k T^-T:

  maha_k(x) = d^T S^-1 d - ||U_k^T d||^2,   d = x - mu_k

Expanding in x, the per-sample log responsibility becomes

  log_resps[n,k] = z[n,k] - 0.5*r[n]
  z[n,k]  = const_k + x_n . g_k + || (U_k/sqrt2)^T x_n ||^2
  r[n]    = x_n^T S^-1 x_n

r cancels in the normalized output; it only shifts the log-likelihood.
The device computes, per 128-row tile of X (X is fed pre-transposed, D on
partitions):
  P   = X @ Wh          (Wh = [U_k/sqrt2] stacked, [128, 512])  - PE, fp32r
  crs = X @ GC + const  (GC = [g_k], [128, 32]; const via rank-1) - PE
  rr  = (X*X) @ (-0.5/s)                                         - PE
  z   = groupsum_16(P^2) + crs                                   - ACT+DVE
then one batched logsumexp over all 20 tiles (single exp / single ln, so
the ACT engine loads its function tables at most twice).
Host does only the O(K*D*Q) parameter factorization (tiny) and the
shard/unshard.  Sharding: data-parallel over N, 8 ways, no collectives.
"""

import json
import os
import shutil
import tempfile

import numpy as np

import concourse.bacc as bacc
import concourse.bass as bass
import concourse.mybir as mybir
import concourse.tile as tile
from concourse.bass_utils import run_bass_kernel_spmd


def _install_act_tables():
    """Reorder the ACT function-table sets so the one set that covers every
    function this kernel uses (ln, exp, square, identity, copy) comes first.
    walrus assigns each ACTIVATE the first set containing its function, so
    this removes all mid-kernel ACT_TABLE_LOAD switches (~1.3us each)."""
    if os.environ.get("BASS_ACT_ROOT_JSON_PATH"):
        return
    try:
        from neuronxcc.driver.Job import Job
        from neuronxcc.driver.jobs.support.FindActInfo import findActInfoFile

        src = findActInfoFile(Job.getPackageDir(), "gen3")
        d = json.load(open(src))
        sets = d["act_func_sets"]
        best = [s for s in sets if s["name"] == "natural_log_exp_and_others"]
        rest = [s for s in sets if s["name"] != "natural_log_exp_and_others"]
        if not best:
            return
        d["act_func_sets"] = best + rest
        dst_dir = tempfile.mkdtemp(prefix="act_tables_")
        for f in os.listdir(os.path.dirname(src)):
            sp = os.path.join(os.path.dirname(src), f)
            if os.path.isfile(sp) and f != "act_info.json":
                os.symlink(sp, os.path.join(dst_dir, f))
        with open(os.path.join(dst_dir, "act_info.json"), "w") as f:
            json.dump(d, f)
        os.environ["BASS_ACT_ROOT_JSON_PATH"] = os.path.join(
            dst_dir, "act_info.json")
    except Exception:
        pass


if os.environ.get("MFA_ACT_TABLES", "0") == "1":
    _install_act_tables()


def _fast_drain_and_barrier(self, tick_clock, wait_clock):
    """Cheap Tile epilogue: the sync drain already waits on the global
    vector clock (all engines + DMA queues complete), so the two all-engine
    EVSEM butterfly barriers (~5us each) reduce to one semaphore handoff:
    sync -> gpsimd, which then clears the tile semaphores for NEFF re-use."""
    from concourse.vector_clock import ScopedClock as _SC

    nc = self.nc
    drain_inst = nc.sync.drain()
    wait_clock.add_sem_waits(
        drain_inst.ins, _SC({None: tick_clock.global_clock})
    )
    done = nc.alloc_semaphore("tail_done")
    nc.sync.sem_inc(done, 1)
    nc.gpsimd.wait_ge(done, 1)
    popped = nc._tile_sem_poison_stack.pop()
    assert popped is self._sem_poison
    assert self.sems is not None
    nc.clear_and_free_semaphores(list(self.sems.allocated().values()))
    nc.gpsimd.sem_clear(done)
    nc.release_semaphore(done)


if os.environ.get("MFA_FAST_TAIL", "1") == "1":
    tile.TileContext._drain_and_barrier = _fast_drain_and_barrier

K, D, Q, N = 32, 128, 16, 20000
NCORES = 8
NPAD = 20480          # N padded to 8 * 2560
NLOC = NPAD // NCORES  # 2560 rows per core
PT = 128               # rows per tile (partition dim)
NT = NLOC // PT        # 20 tiles per core
KQ = K * Q             # 512

F32 = mybir.dt.float32
F32R = mybir.dt.float32r
F16 = mybir.dt.float16
AX = mybir.AxisListType
ALU = mybir.AluOpType
ACTF = mybir.ActivationFunctionType

USE_F32R = True       # main P matmul in fp32r
AUX_F32R = True       # crs / rr matmuls in fp32r
POOL_REDUCE = False   # grouped sum-of-squares via pool_avg instead of reduce
SQ_F16 = False        # P^2 stored as fp16 (no DVE speedup observed; off)
SOFT_LN = True        # ln(ssum) in software on GpSimd; ACT keeps one table
PH2_CHUNKS = 2        # logsumexp phases overlapping the main loop
XCHUNKS = 5           # X DMA + fp32r cast pipelined in this many chunks
PSP_BUFS = 4
SQ_BUFS = 4


def build_bass():
    """Build the per-core Tile program (same NEFF on all 8 cores)."""
    nc = bacc.Bacc("TRN2", target_bir_lowering=False, debug=False)

    # X shard arrives pre-transposed: [D, NLOC], so tiles DMA straight into
    # the matmul operand layout (D on partitions) with no on-chip transpose.
    XsT = nc.dram_tensor("XsT", [D, NLOC], F32, kind="ExternalInput")
    Wh = nc.dram_tensor("Wh", [D, KQ], F32, kind="ExternalInput")
    GC = nc.dram_tensor("GC", [D, K], F32, kind="ExternalInput")
    sneg = nc.dram_tensor("sneg", [D, 2], F32, kind="ExternalInput")
    constb = nc.dram_tensor("constb", [PT, K], F32, kind="ExternalInput")
    out_norm = nc.dram_tensor("out_norm", [NLOC, K], F32, kind="ExternalOutput")
    out_ll = nc.dram_tensor("out_ll", [NLOC, 1], F32, kind="ExternalOutput")

    with tile.TileContext(nc) as tc:
        with (
            tc.tile_pool(name="consts", bufs=1) as cpool,
            tc.tile_pool(name="xbig", bufs=1) as xbig,
            tc.tile_pool(name="sq", bufs=SQ_BUFS) as sq_pool,
            tc.tile_pool(name="acc", bufs=1) as accp,
            tc.tile_pool(name="small", bufs=2) as spool,
            tc.tile_pool(name="pP", bufs=PSP_BUFS,
                         space=bass.MemorySpace.PSUM) as psP_pool,
            tc.tile_pool(name="pC", bufs=2, space=bass.MemorySpace.PSUM) as psC_pool,
            tc.tile_pool(name="pR", bufs=2, space=bass.MemorySpace.PSUM) as psR_pool,
        ):
            def load_const(name, dram, shape, rdt):
                t = cpool.tile(shape, F32, tag=name)
                nc.sync.dma_start(out=t[:], in_=dram[:])
                if rdt == F32:
                    return t
                tr = cpool.tile(shape, F32R, tag=name + "_r")
                nc.vector.tensor_copy(tr[:], t[:])
                return tr

            auxdt = F32R if AUX_F32R else F32
            maindt = F32R if USE_F32R else F32

            # whole X shard in SBUF, transposed layout [D, NLOC]; DMA, fp32r
            # cast, and x^2 all pipelined in XCHUNKS chunks so the first
            # matmul can start early.
            xt_all = xbig.tile([D, NLOC], F32)
            if USE_F32R or AUX_F32R:
                xtr_all = xbig.tile([D, NLOC], F32R)
            else:
                xtr_all = xt_all
            x2_all = xbig.tile([D, NLOC], F32R if AUX_F32R else F32)

            def xchunk(cs):
                nc.sync.dma_start(out=xt_all[:, cs], in_=XsT[:, cs])
                if USE_F32R or AUX_F32R:
                    nc.vector.tensor_copy(xtr_all[:, cs], xt_all[:, cs])
                nc.scalar.square(x2_all[:, cs], xt_all[:, cs])

            # DMA order on the sync queue is the startup critical path:
            # a small X chunk first (gates the first matmul together with
            # wh), then wh, then the rest of X interleaved with the small
            # constants.
            xchunk(slice(0, 2 * PT))
            wh_t = load_const("wh", Wh, [D, KQ], maindt)
            gc_t = load_const("gc", GC, [D, K], auxdt)
            # fp32r matmuls need an even output free size; sneg is [D,2]
            # host-side with a zero second column.
            sneg_t = load_const("sneg", sneg, [D, 2], auxdt)
            constb_t = cpool.tile([PT, K], F32, tag="constb")
            nc.sync.dma_start(out=constb_t[:], in_=constb[:])
            rest = NLOC - 2 * PT
            nch = XCHUNKS - 1
            step = (rest // nch // PT) * PT
            b = 2 * PT
            for c in range(nch):
                e = NLOC if c == nch - 1 else b + step
                xchunk(slice(b, e))
                b = e

            xm_all = xtr_all if USE_F32R else xt_all
            xa_all = xtr_all if AUX_F32R else xt_all

            # accumulators across all tiles
            z_all = accp.tile([PT, NT, K], F32)     # z per (row, tile, k)
            rr_all = accp.tile([PT, NT], F32)       # -0.5 r per (row, tile)
            ev = accp.tile([PT, NT, K], F32)
            outn = accp.tile([PT, NT, K], F32)

            CH = NT // PH2_CHUNKS

            def phase2(c):
                """Batched logsumexp for tiles [c*CH, (c+1)*CH)."""
                ts = slice(c * CH, (c + 1) * CH)
                zf = z_all[:, ts, :].rearrange("p t k -> p (t k)")
                negm = spool.tile([PT, 1], F32, tag="negm")
                nc.vector.tensor_reduce(negm[:], zf, axis=AX.X, op=ALU.max,
                                        negate=True)
                nc.scalar.activation(
                    ev[:, ts, :].rearrange("p t k -> p (t k)"), zf,
                    ACTF.Exp, bias=negm[:, 0:1], scale=1.0)
                ssum = spool.tile([PT, CH], F32, tag="ssum")
                nc.vector.tensor_reduce(ssum[:], ev[:, ts, :], axis=AX.X,
                                        op=ALU.add)
                lg = spool.tile([PT, CH], F32, tag="lg")
                if SOFT_LN:
                    # ln on GpSimd (keeps the ACT engine on a single table
                    # set): ln(y) = (e-127)ln2 + 2*atanh(s), s=(m-1)/(m+1),
                    # atanh(s) ~ s*(1 + s^2/3 + s^4/5), |s|<0.1716.
                    I32 = mybir.dt.int32
                    bits = ssum[:].bitcast(I32)
                    e_i = spool.tile([PT, CH], I32, tag="ln_ei")
                    nc.vector.tensor_scalar(e_i[:], bits, 23, None,
                                            op0=ALU.arith_shift_right)
                    e_f = spool.tile([PT, CH], F32, tag="ln_ef")
                    nc.vector.tensor_copy(e_f[:], e_i[:])   # int -> float
                    LN2 = 0.6931471805599453
                    et = spool.tile([PT, CH], F32, tag="ln_et")
                    nc.vector.tensor_scalar(et[:], e_f[:], LN2, -127.0 * LN2,
                                            op0=ALU.mult, op1=ALU.add)
                    mb = spool.tile([PT, CH], I32, tag="ln_mb")
                    nc.vector.tensor_scalar(mb[:], bits, 0x007FFFFF,
                                            0x3F800000,
                                            op0=ALU.bitwise_and,
                                            op1=ALU.bitwise_or)
                    mant = mb[:].bitcast(F32)
                    num = spool.tile([PT, CH], F32, tag="ln_num")
                    nc.vector.tensor_scalar(num[:], mant, 1.0, None,
                                            op0=ALU.subtract)
                    den = spool.tile([PT, CH], F32, tag="ln_den")
                    nc.vector.tensor_scalar(den[:], mant, 1.0, None,
                                            op0=ALU.add)
                    rden = spool.tile([PT, CH], F32, tag="ln_rden")
                    nc.vector.reciprocal(rden[:], den[:])
                    sv = spool.tile([PT, CH], F32, tag="ln_s")
                    nc.vector.tensor_tensor(sv[:], num[:], rden[:],
                                            op=ALU.mult)
                    s2 = spool.tile([PT, CH], F32, tag="ln_s2")
                    nc.vector.tensor_tensor(s2[:], sv[:], sv[:],
                                            op=ALU.mult)
                    tpoly = spool.tile([PT, CH], F32, tag="ln_t")
                    nc.vector.tensor_scalar(tpoly[:], s2[:], 0.2, None,
                                            op0=ALU.mult)
                    nc.vector.scalar_tensor_tensor(
                        tpoly[:], tpoly[:], 1.0 / 3.0, s2[:],
                        op0=ALU.add, op1=ALU.mult)
                    lnm2 = spool.tile([PT, CH], F32, tag="ln_lnm2")
                    nc.vector.scalar_tensor_tensor(
                        lnm2[:], tpoly[:], 1.0, sv[:],
                        op0=ALU.add, op1=ALU.mult)
                    nc.vector.scalar_tensor_tensor(
                        lg[:], lnm2[:], 2.0, et[:],
                        op0=ALU.mult, op1=ALU.add)
                else:
                    nc.scalar.activation(lg[:], ssum[:], ACTF.Ln)
                # lse[p,t] = lg[p,t] + m[p] = lg - negm
                lse = spool.tile([PT, CH], F32, tag="lse")
                nc.vector.tensor_scalar(lse[:], lg[:], negm[:, 0:1], None,
                                        op0=ALU.subtract)
                # ll = lse + rr
                ll = spool.tile([PT, CH], F32, tag="ll")
                nc.vector.tensor_add(ll[:], lse[:], rr_all[:, ts])
                # outn = z - lse (broadcast along k)
                lse_b = lse[:].unsqueeze(2).broadcast_to([PT, CH, K])
                nc.vector.tensor_sub(outn[:, ts, :], z_all[:, ts, :], lse_b)

                # out_norm[(t*128+p), k] = outn[p, t, k]
                on_view = out_norm.ap().rearrange("(t p) k -> p t k", p=PT)
                nc.sync.dma_start(out=on_view[:, ts, :], in_=outn[:, ts, :])
                oll_view = out_ll.ap().rearrange("(t p) one -> p (t one)",
                                                 p=PT)
                nc.sync.dma_start(out=oll_view[:, ts], in_=ll[:])

            for i in range(NT):
                cols = slice(i * PT, (i + 1) * PT)

                # P = X @ Wh   -> [n, 512]
                psP = psP_pool.tile([PT, KQ], F32, tag="psP")
                nc.tensor.matmul(psP[:], xm_all[:, cols], wh_t[:],
                                 start=True, stop=True)
                # crs = X @ GC -> [n, 32]
                psC = psC_pool.tile([PT, K], F32, tag="psC")
                nc.tensor.matmul(psC[:], xa_all[:, cols], gc_t[:],
                                 start=True, stop=True)
                # rr = (X*X) @ sneg -> [n, 1]
                psR = psR_pool.tile([PT, 2], F32, tag="psR")
                nc.tensor.matmul(psR[:], x2_all[:, cols], sneg_t[:],
                                 start=True, stop=True)

                # sq = P^2 (ACT, PSUM->SBUF)
                sq = sq_pool.tile([PT, KQ], F16 if SQ_F16 else F32, tag="sq")
                nc.scalar.square(sq[:], psP[:])

                # rr slice first, on ACT (DVE is the steady-state
                # bottleneck): frees psR early
                nc.scalar.copy(rr_all[:, i:i + 1], psR[:, 0:1])

                # z0[n,k] = sum_q sq[n, k*16+q] (DVE grouped reduce; fp16
                # in+out lets the DVE run its 2x mode)
                sqg = sq[:].rearrange("p (k q) -> p k q", q=Q)
                z0 = spool.tile([PT, K], F16 if SQ_F16 else F32, tag="z0")
                with nc.allow_low_precision("z0 ~ O(30), fp16 err ~2e-2"):
                    nc.vector.tensor_reduce(z0[:], sqg, axis=AX.X, op=ALU.add)
                # z = z0 + crs (DVE reads PSUM; frees psC), then += const
                nc.vector.tensor_add(z_all[:, i, :], z0[:], psC[:])
                nc.gpsimd.tensor_add(z_all[:, i, :], z_all[:, i, :],
                                     constb_t[:])

                if (i + 1) % CH == 0:
                    phase2(i // CH)

    nc.compile()
    return nc


def host_precompute(X, log_pi, mu, Lam, log_psi):
    """Tiny O(K*D*Q) parameter factorization, in float64 for accuracy."""
    log_pi = np.asarray(log_pi, np.float64)
    mu = np.asarray(mu, np.float64)
    Lam = np.asarray(Lam, np.float64)
    log_psi = np.asarray(log_psi, np.float64)

    s = np.exp(log_psi) + 1e-5 + 1e-4                       # [D]
    sinv = 1.0 / s
    B = Lam * (s ** -0.5)[None, :, None]                    # [K,D,Q]
    M = np.eye(Q)[None] + np.einsum('kdq,kdr->kqr', B, B)   # [K,Q,Q]
    T = np.linalg.cholesky(M)
    logdet = np.sum(np.log(s)) + 2.0 * np.log(
        np.diagonal(T, axis1=1, axis2=2)).sum(1)            # [K]
    Tinv = np.linalg.inv(T)
    U = np.einsum('d,kdq,krq->kdr', sinv, Lam, Tinv)        # [K,D,Q]
    a = sinv[None, :] * mu                                  # [K,D]
    c = np.einsum('kdq,kd->kq', U, mu)                      # [K,Q]
    v = np.einsum('kdq,kq->kd', U, c)                       # [K,D]
    g = a - v                                               # [K,D]
    q1 = np.einsum('kd,kd->k', mu, a)
    q2 = np.einsum('kq,kq->k', c, c)
    const = (log_pi - 0.5 * (D * np.log(2 * np.pi) + logdet)
             - 0.5 * q1 + 0.5 * q2)                         # [K]

    # scale so that the device's grouped reduce (plain sum, or avg-pool which
    # divides by Q) yields exactly 0.5 * ||U^T x||^2
    wscale = np.sqrt(Q / 2.0) if POOL_REDUCE else np.sqrt(0.5)
    Wh = (U * wscale).transpose(0, 2, 1).reshape(KQ, D).T  # [D, KQ]
    return {
        "Wh": np.ascontiguousarray(Wh, dtype=np.float32),
        "GC": np.ascontiguousarray(g.T, dtype=np.float32),
        "sneg": np.ascontiguousarray(
            np.stack([-0.5 * sinv, np.zeros(D)], axis=1), dtype=np.float32),
        "constb": np.ascontiguousarray(
            np.broadcast_to(const[None, :], (PT, K)), dtype=np.float32),
    }


_NC_CACHE = None


def get_nc():
    global _NC_CACHE
    if _NC_CACHE is None:
        _NC_CACHE = build_bass()
    return _NC_CACHE


def kernel(X, log_pi, mu, Lam, log_psi, _collect=None):
    X = np.asarray(X, np.float32)
    params = host_precompute(X, log_pi, mu, Lam, log_psi)

    Xpad = np.zeros((NPAD, D), dtype=np.float32)
    Xpad[:N] = X
    # per-core transposed shards [D, NLOC]
    shards = Xpad.reshape(NCORES, NLOC, D)

    in_maps = [dict(params, XsT=np.ascontiguousarray(shards[c].T))
               for c in range(NCORES)]

    nc = get_nc()
    res = run_bass_kernel_spmd(nc, in_maps, list(range(NCORES)),
                               **(_collect or {}))
    if _collect is not None:
        _collect["res"] = res

    # device row order within a core is (t*128 + p); it matches the shard's
    # natural row order, so plain concatenation restores global order.
    norm = np.concatenate([res.results[c]["out_norm"] for c in range(NCORES)],
                          axis=0)[:N]
    ll = np.concatenate([res.results[c]["out_ll"] for c in range(NCORES)],
                        axis=0)[:N, 0]
    return norm, ll


# revision 42
# speedup vs baseline: 1.5033x; 1.2786x over previous
"""MFA e-step (mixture of factor analyzers) on 8 Trainium2 NeuronCores.

Math: the reference computes per-component Gaussian log-likelihoods with
covariance C_k = Lam_k Lam_k^T + diag(psi).  Since Q=16 << D=128 we use the
Woodbury identity: with S = diag(psi), M_k = I + Lam_k^T S^-1 Lam_k = T T^T,
U_k = S^-1 Lam_k T^-T:

  maha_k(x) = d^T S^-1 d - ||U_k^T d||^2,   d = x - mu_k

Expanding in x, the per-sample log responsibility becomes

  log_resps[n,k] = z[n,k] - 0.5*r[n]
  z[n,k]  = const_k + x_n . g_k + || (U_k/sqrt2)^T x_n ||^2
  r[n]    = x_n^T S^-1 x_n

r cancels in the normalized output; it only shifts the log-likelihood.
The device computes, per 128-row tile of X (X is fed pre-transposed, D on
partitions):
  P   = X @ Wh          (Wh = [U_k/sqrt2] stacked, [128, 512])  - PE, fp32r
  crs = X @ GC + const  (GC = [g_k], [128, 32]; const via rank-1) - PE
  rr  = (X*X) @ (-0.5/s)                                         - PE
  z   = groupsum_16(P^2) + crs                                   - ACT+DVE
then one batched logsumexp over all 20 tiles (single exp / single ln, so
the ACT engine loads its function tables at most twice).
Host does only the O(K*D*Q) parameter factorization (tiny) and the
shard/unshard.  Sharding: data-parallel over N, 8 ways, no collectives.
"""

import json
import os
import shutil
import tempfile

import numpy as np

import concourse.bacc as bacc
import concourse.bass as bass
import concourse.mybir as mybir
import concourse.tile as tile
from concourse.bass_utils import run_bass_kernel_spmd


def _install_act_tables():
    """Reorder the ACT function-table sets so the one set that covers every
    function this kernel uses (ln, exp, square, identity, copy) comes first.
    walrus assigns each ACTIVATE the first set containing its function, so
    this removes all mid-kernel ACT_TABLE_LOAD switches (~1.3us each)."""
    if os.environ.get("BASS_ACT_ROOT_JSON_PATH"):
        return
    try:
        from neuronxcc.driver.Job import Job
        from neuronxcc.driver.jobs.support.FindActInfo import findActInfoFile

        src = findActInfoFile(Job.getPackageDir(), "gen3")
        d = json.load(open(src))
        sets = d["act_func_sets"]
        best = [s for s in sets if s["name"] == "natural_log_exp_and_others"]
        rest = [s for s in sets if s["name"] != "natural_log_exp_and_others"]
        if not best:
            return
        d["act_func_sets"] = best + rest
        dst_dir = tempfile.mkdtemp(prefix="act_tables_")
        for f in os.listdir(os.path.dirname(src)):
            sp = os.path.join(os.path.dirname(src), f)
            if os.path.isfile(sp) and f != "act_info.json":
                os.symlink(sp, os.path.join(dst_dir, f))
        with open(os.path.join(dst_dir, "act_info.json"), "w") as f:
            json.dump(d, f)
        os.environ["BASS_ACT_ROOT_JSON_PATH"] = os.path.join(
            dst_dir, "act_info.json")
    except Exception:
        pass


if os.environ.get("MFA_ACT_TABLES", "0") == "1":
    _install_act_tables()


def _fast_drain_and_barrier(self, tick_clock, wait_clock):
    """Cheap Tile epilogue: the sync drain already waits on the global
    vector clock (all engines + DMA queues complete), so the two all-engine
    EVSEM butterfly barriers (~5us each) reduce to one semaphore handoff:
    sync -> gpsimd, which then clears the tile semaphores for NEFF re-use."""
    from concourse.vector_clock import ScopedClock as _SC

    nc = self.nc
    drain_inst = nc.sync.drain()
    wait_clock.add_sem_waits(
        drain_inst.ins, _SC({None: tick_clock.global_clock})
    )
    done = nc.alloc_semaphore("tail_done")
    nc.sync.sem_inc(done, 1)
    nc.gpsimd.wait_ge(done, 1)
    popped = nc._tile_sem_poison_stack.pop()
    assert popped is self._sem_poison
    assert self.sems is not None
    nc.clear_and_free_semaphores(list(self.sems.allocated().values()))
    nc.gpsimd.sem_clear(done)
    nc.release_semaphore(done)


if os.environ.get("MFA_FAST_TAIL", "1") == "1":
    tile.TileContext._drain_and_barrier = _fast_drain_and_barrier

K, D, Q, N = 32, 128, 16, 20000
NCORES = 8
NPAD = 20480          # N padded to 8 * 2560
NLOC = NPAD // NCORES  # 2560 rows per core
PT = 128               # rows per tile (partition dim)
NT = NLOC // PT        # 20 tiles per core
KQ = K * Q             # 512

F32 = mybir.dt.float32
F32R = mybir.dt.float32r
F16 = mybir.dt.float16
AX = mybir.AxisListType
ALU = mybir.AluOpType
ACTF = mybir.ActivationFunctionType

USE_F32R = True       # main P matmul in fp32r
AUX_F32R = True       # crs / rr matmuls in fp32r
POOL_REDUCE = False   # grouped sum-of-squares via pool_avg instead of reduce
SQ_F16 = False        # P^2 stored as fp16 (no DVE speedup observed; off)
SOFT_LN = True        # ln(ssum) in software on GpSimd; ACT keeps one table
PH2_CHUNKS = 2        # logsumexp phases overlapping the main loop
XCHUNKS = 5           # X DMA + fp32r cast pipelined in this many chunks
PSP_BUFS = 4
SQ_BUFS = 4


def build_bass():
    """Build the per-core Tile program (same NEFF on all 8 cores)."""
    nc = bacc.Bacc("TRN2", target_bir_lowering=False, debug=False)

    # X shard arrives pre-transposed: [D, NLOC], so tiles DMA straight into
    # the matmul operand layout (D on partitions) with no on-chip transpose.
    XsT = nc.dram_tensor("XsT", [D, NLOC], F32, kind="ExternalInput")
    Wh = nc.dram_tensor("Wh", [D, KQ], F32, kind="ExternalInput")
    GC = nc.dram_tensor("GC", [D, K], F32, kind="ExternalInput")
    sneg = nc.dram_tensor("sneg", [D, 2], F32, kind="ExternalInput")
    constb = nc.dram_tensor("constb", [PT, K], F32, kind="ExternalInput")
    # outputs stay in the device-natural [p, t, k] layout so the final DMA
    # is 128 fat contiguous descriptors instead of 2560 thin ones; the host
    # undoes the (t p) interleave with a cheap transpose.
    out_norm = nc.dram_tensor("out_norm", [PT, NT, K], F32, kind="ExternalOutput")
    out_ll = nc.dram_tensor("out_ll", [PT, NT], F32, kind="ExternalOutput")

    with tile.TileContext(nc) as tc:
        with (
            tc.tile_pool(name="consts", bufs=1) as cpool,
            tc.tile_pool(name="xbig", bufs=1) as xbig,
            tc.tile_pool(name="sq", bufs=SQ_BUFS) as sq_pool,
            tc.tile_pool(name="acc", bufs=1) as accp,
            tc.tile_pool(name="small", bufs=2) as spool,
            tc.tile_pool(name="pP", bufs=PSP_BUFS,
                         space=bass.MemorySpace.PSUM) as psP_pool,
            tc.tile_pool(name="pC", bufs=2, space=bass.MemorySpace.PSUM) as psC_pool,
            tc.tile_pool(name="pR", bufs=2, space=bass.MemorySpace.PSUM) as psR_pool,
        ):
            def load_const(name, dram, shape, rdt):
                t = cpool.tile(shape, F32, tag=name)
                nc.sync.dma_start(out=t[:], in_=dram[:])
                if rdt == F32:
                    return t
                tr = cpool.tile(shape, F32R, tag=name + "_r")
                nc.vector.tensor_copy(tr[:], t[:])
                return tr

            auxdt = F32R if AUX_F32R else F32
            maindt = F32R if USE_F32R else F32

            # whole X shard in SBUF, transposed layout [D, NLOC]; DMA, fp32r
            # cast, and x^2 all pipelined in XCHUNKS chunks so the first
            # matmul can start early.
            xt_all = xbig.tile([D, NLOC], F32)
            if USE_F32R or AUX_F32R:
                xtr_all = xbig.tile([D, NLOC], F32R)
            else:
                xtr_all = xt_all
            x2_all = xbig.tile([D, NLOC], F32R if AUX_F32R else F32)

            def xchunk(cs):
                nc.sync.dma_start(out=xt_all[:, cs], in_=XsT[:, cs])
                if USE_F32R or AUX_F32R:
                    nc.vector.tensor_copy(xtr_all[:, cs], xt_all[:, cs])
                nc.scalar.square(x2_all[:, cs], xt_all[:, cs])

            # DMA order on the sync queue is the startup critical path:
            # a small X chunk first (gates the first matmul together with
            # wh), then wh, then the rest of X interleaved with the small
            # constants.
            xchunk(slice(0, 2 * PT))
            wh_t = load_const("wh", Wh, [D, KQ], maindt)
            gc_t = load_const("gc", GC, [D, K], auxdt)
            # fp32r matmuls need an even output free size; sneg is [D,2]
            # host-side with a zero second column.
            sneg_t = load_const("sneg", sneg, [D, 2], auxdt)
            constb_t = cpool.tile([PT, K], F32, tag="constb")
            nc.sync.dma_start(out=constb_t[:], in_=constb[:])
            rest = NLOC - 2 * PT
            nch = XCHUNKS - 1
            step = (rest // nch // PT) * PT
            b = 2 * PT
            for c in range(nch):
                e = NLOC if c == nch - 1 else b + step
                xchunk(slice(b, e))
                b = e

            xm_all = xtr_all if USE_F32R else xt_all
            xa_all = xtr_all if AUX_F32R else xt_all

            # accumulators across all tiles
            z_all = accp.tile([PT, NT, K], F32)     # z per (row, tile, k)
            rr_all = accp.tile([PT, NT], F32)       # -0.5 r per (row, tile)
            ev = accp.tile([PT, NT, K], F32)
            outn = accp.tile([PT, NT, K], F32)

            CH = NT // PH2_CHUNKS

            def phase2(c):
                """Batched logsumexp for tiles [c*CH, (c+1)*CH)."""
                ts = slice(c * CH, (c + 1) * CH)
                zf = z_all[:, ts, :].rearrange("p t k -> p (t k)")
                negm = spool.tile([PT, 1], F32, tag="negm")
                nc.vector.tensor_reduce(negm[:], zf, axis=AX.X, op=ALU.max,
                                        negate=True)
                nc.scalar.activation(
                    ev[:, ts, :].rearrange("p t k -> p (t k)"), zf,
                    ACTF.Exp, bias=negm[:, 0:1], scale=1.0)
                ssum = spool.tile([PT, CH], F32, tag="ssum")
                nc.vector.tensor_reduce(ssum[:], ev[:, ts, :], axis=AX.X,
                                        op=ALU.add)
                lg = spool.tile([PT, CH], F32, tag="lg")
                if SOFT_LN:
                    # ln on GpSimd (keeps the ACT engine on a single table
                    # set): ln(y) = (e-127)ln2 + 2*atanh(s), s=(m-1)/(m+1),
                    # atanh(s) ~ s*(1 + s^2/3 + s^4/5), |s|<0.1716.
                    I32 = mybir.dt.int32
                    bits = ssum[:].bitcast(I32)
                    e_i = spool.tile([PT, CH], I32, tag="ln_ei")
                    nc.vector.tensor_scalar(e_i[:], bits, 23, None,
                                            op0=ALU.arith_shift_right)
                    e_f = spool.tile([PT, CH], F32, tag="ln_ef")
                    nc.vector.tensor_copy(e_f[:], e_i[:])   # int -> float
                    LN2 = 0.6931471805599453
                    et = spool.tile([PT, CH], F32, tag="ln_et")
                    nc.vector.tensor_scalar(et[:], e_f[:], LN2, -127.0 * LN2,
                                            op0=ALU.mult, op1=ALU.add)
                    mb = spool.tile([PT, CH], I32, tag="ln_mb")
                    nc.vector.tensor_scalar(mb[:], bits, 0x007FFFFF,
                                            0x3F800000,
                                            op0=ALU.bitwise_and,
                                            op1=ALU.bitwise_or)
                    mant = mb[:].bitcast(F32)
                    num = spool.tile([PT, CH], F32, tag="ln_num")
                    nc.vector.tensor_scalar(num[:], mant, 1.0, None,
                                            op0=ALU.subtract)
                    den = spool.tile([PT, CH], F32, tag="ln_den")
                    nc.vector.tensor_scalar(den[:], mant, 1.0, None,
                                            op0=ALU.add)
                    rden = spool.tile([PT, CH], F32, tag="ln_rden")
                    nc.vector.reciprocal(rden[:], den[:])
                    sv = spool.tile([PT, CH], F32, tag="ln_s")
                    nc.vector.tensor_tensor(sv[:], num[:], rden[:],
                                            op=ALU.mult)
                    s2 = spool.tile([PT, CH], F32, tag="ln_s2")
                    nc.vector.tensor_tensor(s2[:], sv[:], sv[:],
                                            op=ALU.mult)
                    tpoly = spool.tile([PT, CH], F32, tag="ln_t")
                    nc.vector.tensor_scalar(tpoly[:], s2[:], 0.2, None,
                                            op0=ALU.mult)
                    nc.vector.scalar_tensor_tensor(
                        tpoly[:], tpoly[:], 1.0 / 3.0, s2[:],
                        op0=ALU.add, op1=ALU.mult)
                    lnm2 = spool.tile([PT, CH], F32, tag="ln_lnm2")
                    nc.vector.scalar_tensor_tensor(
                        lnm2[:], tpoly[:], 1.0, sv[:],
                        op0=ALU.add, op1=ALU.mult)
                    nc.vector.scalar_tensor_tensor(
                        lg[:], lnm2[:], 2.0, et[:],
                        op0=ALU.mult, op1=ALU.add)
                else:
                    nc.scalar.activation(lg[:], ssum[:], ACTF.Ln)
                # lse[p,t] = lg[p,t] + m[p] = lg - negm
                lse = spool.tile([PT, CH], F32, tag="lse")
                nc.vector.tensor_scalar(lse[:], lg[:], negm[:, 0:1], None,
                                        op0=ALU.subtract)
                # ll = lse + rr
                ll = spool.tile([PT, CH], F32, tag="ll")
                nc.vector.tensor_add(ll[:], lse[:], rr_all[:, ts])
                # outn = z - lse (broadcast along k)
                lse_b = lse[:].unsqueeze(2).broadcast_to([PT, CH, K])
                nc.vector.tensor_sub(outn[:, ts, :], z_all[:, ts, :], lse_b)

                nc.sync.dma_start(out=out_norm.ap()[:, ts, :],
                                  in_=outn[:, ts, :])
                nc.sync.dma_start(out=out_ll.ap()[:, ts], in_=ll[:])

            for i in range(NT):
                cols = slice(i * PT, (i + 1) * PT)

                # P = X @ Wh   -> [n, 512]
                psP = psP_pool.tile([PT, KQ], F32, tag="psP")
                nc.tensor.matmul(psP[:], xm_all[:, cols], wh_t[:],
                                 start=True, stop=True)
                # crs = X @ GC -> [n, 32]
                psC = psC_pool.tile([PT, K], F32, tag="psC")
                nc.tensor.matmul(psC[:], xa_all[:, cols], gc_t[:],
                                 start=True, stop=True)
                # rr = (X*X) @ sneg -> [n, 1]
                psR = psR_pool.tile([PT, 2], F32, tag="psR")
                nc.tensor.matmul(psR[:], x2_all[:, cols], sneg_t[:],
                                 start=True, stop=True)

                # sq = P^2 (ACT, PSUM->SBUF)
                sq = sq_pool.tile([PT, KQ], F16 if SQ_F16 else F32, tag="sq")
                nc.scalar.square(sq[:], psP[:])

                # rr slice first, on ACT (DVE is the steady-state
                # bottleneck): frees psR early
                nc.scalar.copy(rr_all[:, i:i + 1], psR[:, 0:1])

                # z0[n,k] = sum_q sq[n, k*16+q] (DVE grouped reduce; fp16
                # in+out lets the DVE run its 2x mode)
                sqg = sq[:].rearrange("p (k q) -> p k q", q=Q)
                z0 = spool.tile([PT, K], F16 if SQ_F16 else F32, tag="z0")
                with nc.allow_low_precision("z0 ~ O(30), fp16 err ~2e-2"):
                    nc.vector.tensor_reduce(z0[:], sqg, axis=AX.X, op=ALU.add)
                # z = z0 + crs (DVE reads PSUM; frees psC), then += const
                nc.vector.tensor_add(z_all[:, i, :], z0[:], psC[:])
                nc.gpsimd.tensor_add(z_all[:, i, :], z_all[:, i, :],
                                     constb_t[:])

                if (i + 1) % CH == 0:
                    phase2(i // CH)

    nc.compile()
    return nc


def host_precompute(X, log_pi, mu, Lam, log_psi):
    """Tiny O(K*D*Q) parameter factorization, in float64 for accuracy."""
    log_pi = np.asarray(log_pi, np.float64)
    mu = np.asarray(mu, np.float64)
    Lam = np.asarray(Lam, np.float64)
    log_psi = np.asarray(log_psi, np.float64)

    s = np.exp(log_psi) + 1e-5 + 1e-4                       # [D]
    sinv = 1.0 / s
    B = Lam * (s ** -0.5)[None, :, None]                    # [K,D,Q]
    M = np.eye(Q)[None] + np.einsum('kdq,kdr->kqr', B, B)   # [K,Q,Q]
    T = np.linalg.cholesky(M)
    logdet = np.sum(np.log(s)) + 2.0 * np.log(
        np.diagonal(T, axis1=1, axis2=2)).sum(1)            # [K]
    Tinv = np.linalg.inv(T)
    U = np.einsum('d,kdq,krq->kdr', sinv, Lam, Tinv)        # [K,D,Q]
    a = sinv[None, :] * mu                                  # [K,D]
    c = np.einsum('kdq,kd->kq', U, mu)                      # [K,Q]
    v = np.einsum('kdq,kq->kd', U, c)                       # [K,D]
    g = a - v                                               # [K,D]
    q1 = np.einsum('kd,kd->k', mu, a)
    q2 = np.einsum('kq,kq->k', c, c)
    const = (log_pi - 0.5 * (D * np.log(2 * np.pi) + logdet)
             - 0.5 * q1 + 0.5 * q2)                         # [K]

    # scale so that the device's grouped reduce (plain sum, or avg-pool which
    # divides by Q) yields exactly 0.5 * ||U^T x||^2
    wscale = np.sqrt(Q / 2.0) if POOL_REDUCE else np.sqrt(0.5)
    Wh = (U * wscale).transpose(0, 2, 1).reshape(KQ, D).T  # [D, KQ]
    return {
        "Wh": np.ascontiguousarray(Wh, dtype=np.float32),
        "GC": np.ascontiguousarray(g.T, dtype=np.float32),
        "sneg": np.ascontiguousarray(
            np.stack([-0.5 * sinv, np.zeros(D)], axis=1), dtype=np.float32),
        "constb": np.ascontiguousarray(
            np.broadcast_to(const[None, :], (PT, K)), dtype=np.float32),
    }


_NC_CACHE = None


def get_nc():
    global _NC_CACHE
    if _NC_CACHE is None:
        _NC_CACHE = build_bass()
    return _NC_CACHE


def kernel(X, log_pi, mu, Lam, log_psi, _collect=None):
    X = np.asarray(X, np.float32)
    params = host_precompute(X, log_pi, mu, Lam, log_psi)

    Xpad = np.zeros((NPAD, D), dtype=np.float32)
    Xpad[:N] = X
    # per-core transposed shards [D, NLOC]
    shards = Xpad.reshape(NCORES, NLOC, D)

    in_maps = [dict(params, XsT=np.ascontiguousarray(shards[c].T))
               for c in range(NCORES)]

    nc = get_nc()
    res = run_bass_kernel_spmd(nc, in_maps, list(range(NCORES)),
                               **(_collect or {}))
    if _collect is not None:
        _collect["res"] = res

    # device emits [p, t, k]; shard row n = t*128 + p
    norm = np.concatenate(
        [res.results[c]["out_norm"].transpose(1, 0, 2).reshape(NLOC, K)
         for c in range(NCORES)], axis=0)[:N]
    ll = np.concatenate(
        [res.results[c]["out_ll"].T.reshape(NLOC)
         for c in range(NCORES)], axis=0)[:N]
    return norm, ll


# revision 43
# speedup vs baseline: 1.5153x; 1.0080x over previous
"""MFA e-step (mixture of factor analyzers) on 8 Trainium2 NeuronCores.

Math: the reference computes per-component Gaussian log-likelihoods with
covariance C_k = Lam_k Lam_k^T + diag(psi).  Since Q=16 << D=128 we use the
Woodbury identity: with S = diag(psi), M_k = I + Lam_k^T S^-1 Lam_k = T T^T,
U_k = S^-1 Lam_k T^-T:

  maha_k(x) = d^T S^-1 d - ||U_k^T d||^2,   d = x - mu_k

Expanding in x, the per-sample log responsibility becomes

  log_resps[n,k] = z[n,k] - 0.5*r[n]
  z[n,k]  = const_k + x_n . g_k + || (U_k/sqrt2)^T x_n ||^2
  r[n]    = x_n^T S^-1 x_n

r cancels in the normalized output; it only shifts the log-likelihood.
The device computes, per 128-row tile of X (X is fed pre-transposed, D on
partitions):
  P   = X @ Wh          (Wh = [U_k/sqrt2] stacked, [128, 512])  - PE, fp32r
  crs = X @ GC + const  (GC = [g_k], [128, 32]; const via rank-1) - PE
  rr  = (X*X) @ (-0.5/s)                                         - PE
  z   = groupsum_16(P^2) + crs                                   - ACT+DVE
then one batched logsumexp over all 20 tiles (single exp / single ln, so
the ACT engine loads its function tables at most twice).
Host does only the O(K*D*Q) parameter factorization (tiny) and the
shard/unshard.  Sharding: data-parallel over N, 8 ways, no collectives.
"""

import json
import os
import shutil
import tempfile

import numpy as np

import concourse.bacc as bacc
import concourse.bass as bass
import concourse.mybir as mybir
import concourse.tile as tile
from concourse.bass_utils import run_bass_kernel_spmd


def _install_act_tables():
    """Reorder the ACT function-table sets so the one set that covers every
    function this kernel uses (ln, exp, square, identity, copy) comes first.
    walrus assigns each ACTIVATE the first set containing its function, so
    this removes all mid-kernel ACT_TABLE_LOAD switches (~1.3us each)."""
    if os.environ.get("BASS_ACT_ROOT_JSON_PATH"):
        return
    try:
        from neuronxcc.driver.Job import Job
        from neuronxcc.driver.jobs.support.FindActInfo import findActInfoFile

        src = findActInfoFile(Job.getPackageDir(), "gen3")
        d = json.load(open(src))
        sets = d["act_func_sets"]
        best = [s for s in sets if s["name"] == "natural_log_exp_and_others"]
        rest = [s for s in sets if s["name"] != "natural_log_exp_and_others"]
        if not best:
            return
        d["act_func_sets"] = best + rest
        dst_dir = tempfile.mkdtemp(prefix="act_tables_")
        for f in os.listdir(os.path.dirname(src)):
            sp = os.path.join(os.path.dirname(src), f)
            if os.path.isfile(sp) and f != "act_info.json":
                os.symlink(sp, os.path.join(dst_dir, f))
        with open(os.path.join(dst_dir, "act_info.json"), "w") as f:
            json.dump(d, f)
        os.environ["BASS_ACT_ROOT_JSON_PATH"] = os.path.join(
            dst_dir, "act_info.json")
    except Exception:
        pass


if os.environ.get("MFA_ACT_TABLES", "0") == "1":
    _install_act_tables()


def _fast_drain_and_barrier(self, tick_clock, wait_clock):
    """Cheap Tile epilogue: the sync drain already waits on the global
    vector clock (all engines + DMA queues complete), so the two all-engine
    EVSEM butterfly barriers (~5us each) reduce to one semaphore handoff:
    sync -> gpsimd, which then clears the tile semaphores for NEFF re-use."""
    from concourse.vector_clock import ScopedClock as _SC

    nc = self.nc
    drain_inst = nc.sync.drain()
    wait_clock.add_sem_waits(
        drain_inst.ins, _SC({None: tick_clock.global_clock})
    )
    done = nc.alloc_semaphore("tail_done")
    nc.sync.sem_inc(done, 1)
    nc.gpsimd.wait_ge(done, 1)
    popped = nc._tile_sem_poison_stack.pop()
    assert popped is self._sem_poison
    assert self.sems is not None
    nc.clear_and_free_semaphores(list(self.sems.allocated().values()))
    nc.gpsimd.sem_clear(done)
    nc.release_semaphore(done)


if os.environ.get("MFA_FAST_TAIL", "1") == "1":
    tile.TileContext._drain_and_barrier = _fast_drain_and_barrier


def _install_ldw_opt():
    """Turn walrus's LDWEIGHTS-elision pass back on: consecutive matmuls in
    this kernel reuse the same stationary operand, and walrus's built-in
    birsim golden check still validates the NEFF."""
    import concourse.bass_utils as _bu

    orig = _bu.run_command

    def patched(argv, **kw):
        argv = ["--enable-ldw-opt=true" if a == "--enable-ldw-opt=false"
                else a for a in argv]
        return orig(argv, **kw)

    _bu.run_command = patched


if os.environ.get("MFA_LDW_OPT", "1") == "1":
    _install_ldw_opt()

K, D, Q, N = 32, 128, 16, 20000
NCORES = 8
NPAD = 20480          # N padded to 8 * 2560
NLOC = NPAD // NCORES  # 2560 rows per core
PT = 128               # rows per tile (partition dim)
NT = NLOC // PT        # 20 tiles per core
KQ = K * Q             # 512

F32 = mybir.dt.float32
F32R = mybir.dt.float32r
F16 = mybir.dt.float16
AX = mybir.AxisListType
ALU = mybir.AluOpType
ACTF = mybir.ActivationFunctionType

USE_F32R = True       # main P matmul in fp32r
AUX_F32R = True       # crs / rr matmuls in fp32r
POOL_REDUCE = False   # grouped sum-of-squares via pool_avg instead of reduce
SQ_F16 = False        # P^2 stored as fp16 (no DVE speedup observed; off)
SOFT_LN = True        # ln(ssum) in software on GpSimd; ACT keeps one table
PH2_CHUNKS = 2        # logsumexp phases overlapping the main loop
XCHUNKS = 5           # X DMA + fp32r cast pipelined in this many chunks
PSP_BUFS = 4
SQ_BUFS = 4


def build_bass():
    """Build the per-core Tile program (same NEFF on all 8 cores)."""
    nc = bacc.Bacc("TRN2", target_bir_lowering=False, debug=False)

    # X shard arrives pre-transposed: [D, NLOC], so tiles DMA straight into
    # the matmul operand layout (D on partitions) with no on-chip transpose.
    XsT = nc.dram_tensor("XsT", [D, NLOC], F32, kind="ExternalInput")
    Wh = nc.dram_tensor("Wh", [D, KQ], F32, kind="ExternalInput")
    GC = nc.dram_tensor("GC", [D, K], F32, kind="ExternalInput")
    sneg = nc.dram_tensor("sneg", [D, 2], F32, kind="ExternalInput")
    constb = nc.dram_tensor("constb", [PT, K], F32, kind="ExternalInput")
    # outputs stay in the device-natural [p, t, k] layout so the final DMA
    # is 128 fat contiguous descriptors instead of 2560 thin ones; the host
    # undoes the (t p) interleave with a cheap transpose.
    out_norm = nc.dram_tensor("out_norm", [PT, NT, K], F32, kind="ExternalOutput")
    out_ll = nc.dram_tensor("out_ll", [PT, NT], F32, kind="ExternalOutput")

    with tile.TileContext(nc) as tc:
        with (
            tc.tile_pool(name="consts", bufs=1) as cpool,
            tc.tile_pool(name="xbig", bufs=1) as xbig,
            tc.tile_pool(name="sq", bufs=SQ_BUFS) as sq_pool,
            tc.tile_pool(name="acc", bufs=1) as accp,
            tc.tile_pool(name="small", bufs=2) as spool,
            tc.tile_pool(name="pP", bufs=PSP_BUFS,
                         space=bass.MemorySpace.PSUM) as psP_pool,
            tc.tile_pool(name="pC", bufs=2, space=bass.MemorySpace.PSUM) as psC_pool,
            tc.tile_pool(name="pR", bufs=2, space=bass.MemorySpace.PSUM) as psR_pool,
        ):
            def load_const(name, dram, shape, rdt):
                t = cpool.tile(shape, F32, tag=name)
                nc.sync.dma_start(out=t[:], in_=dram[:])
                if rdt == F32:
                    return t
                tr = cpool.tile(shape, F32R, tag=name + "_r")
                nc.vector.tensor_copy(tr[:], t[:])
                return tr

            auxdt = F32R if AUX_F32R else F32
            maindt = F32R if USE_F32R else F32

            # whole X shard in SBUF, transposed layout [D, NLOC]; DMA, fp32r
            # cast, and x^2 all pipelined in XCHUNKS chunks so the first
            # matmul can start early.
            xt_all = xbig.tile([D, NLOC], F32)
            if USE_F32R or AUX_F32R:
                xtr_all = xbig.tile([D, NLOC], F32R)
            else:
                xtr_all = xt_all
            x2_all = xbig.tile([D, NLOC], F32R if AUX_F32R else F32)

            def xchunk(cs):
                nc.sync.dma_start(out=xt_all[:, cs], in_=XsT[:, cs])
                if USE_F32R or AUX_F32R:
                    nc.vector.tensor_copy(xtr_all[:, cs], xt_all[:, cs])
                nc.scalar.square(x2_all[:, cs], xt_all[:, cs])

            # DMA order on the sync queue is the startup critical path:
            # a small X chunk first (gates the first matmul together with
            # wh), then wh, then the rest of X interleaved with the small
            # constants.
            xchunk(slice(0, 2 * PT))
            wh_t = load_const("wh", Wh, [D, KQ], maindt)
            gc_t = load_const("gc", GC, [D, K], auxdt)
            # fp32r matmuls need an even output free size; sneg is [D,2]
            # host-side with a zero second column.
            sneg_t = load_const("sneg", sneg, [D, 2], auxdt)
            constb_t = cpool.tile([PT, K], F32, tag="constb")
            nc.sync.dma_start(out=constb_t[:], in_=constb[:])
            rest = NLOC - 2 * PT
            nch = XCHUNKS - 1
            step = (rest // nch // PT) * PT
            b = 2 * PT
            for c in range(nch):
                e = NLOC if c == nch - 1 else b + step
                xchunk(slice(b, e))
                b = e

            xm_all = xtr_all if USE_F32R else xt_all
            xa_all = xtr_all if AUX_F32R else xt_all

            # accumulators across all tiles
            z_all = accp.tile([PT, NT, K], F32)     # z per (row, tile, k)
            rr_all = accp.tile([PT, NT], F32)       # -0.5 r per (row, tile)
            ev = accp.tile([PT, NT, K], F32)
            outn = accp.tile([PT, NT, K], F32)

            CH = NT // PH2_CHUNKS

            def phase2(c):
                """Batched logsumexp for tiles [c*CH, (c+1)*CH)."""
                ts = slice(c * CH, (c + 1) * CH)
                zf = z_all[:, ts, :].rearrange("p t k -> p (t k)")
                negm = spool.tile([PT, 1], F32, tag="negm")
                nc.vector.tensor_reduce(negm[:], zf, axis=AX.X, op=ALU.max,
                                        negate=True)
                nc.scalar.activation(
                    ev[:, ts, :].rearrange("p t k -> p (t k)"), zf,
                    ACTF.Exp, bias=negm[:, 0:1], scale=1.0)
                ssum = spool.tile([PT, CH], F32, tag="ssum")
                nc.vector.tensor_reduce(ssum[:], ev[:, ts, :], axis=AX.X,
                                        op=ALU.add)
                lg = spool.tile([PT, CH], F32, tag="lg")
                if SOFT_LN:
                    # ln on GpSimd (keeps the ACT engine on a single table
                    # set): ln(y) = (e-127)ln2 + 2*atanh(s), s=(m-1)/(m+1),
                    # atanh(s) ~ s*(1 + s^2/3 + s^4/5), |s|<0.1716.
                    I32 = mybir.dt.int32
                    bits = ssum[:].bitcast(I32)
                    e_i = spool.tile([PT, CH], I32, tag="ln_ei")
                    nc.vector.tensor_scalar(e_i[:], bits, 23, None,
                                            op0=ALU.arith_shift_right)
                    e_f = spool.tile([PT, CH], F32, tag="ln_ef")
                    nc.vector.tensor_copy(e_f[:], e_i[:])   # int -> float
                    LN2 = 0.6931471805599453
                    et = spool.tile([PT, CH], F32, tag="ln_et")
                    nc.vector.tensor_scalar(et[:], e_f[:], LN2, -127.0 * LN2,
                                            op0=ALU.mult, op1=ALU.add)
                    mb = spool.tile([PT, CH], I32, tag="ln_mb")
                    nc.vector.tensor_scalar(mb[:], bits, 0x007FFFFF,
                                            0x3F800000,
                                            op0=ALU.bitwise_and,
                                            op1=ALU.bitwise_or)
                    mant = mb[:].bitcast(F32)
                    num = spool.tile([PT, CH], F32, tag="ln_num")
                    nc.vector.tensor_scalar(num[:], mant, 1.0, None,
                                            op0=ALU.subtract)
                    den = spool.tile([PT, CH], F32, tag="ln_den")
                    nc.vector.tensor_scalar(den[:], mant, 1.0, None,
                                            op0=ALU.add)
                    rden = spool.tile([PT, CH], F32, tag="ln_rden")
                    nc.vector.reciprocal(rden[:], den[:])
                    sv = spool.tile([PT, CH], F32, tag="ln_s")
                    nc.vector.tensor_tensor(sv[:], num[:], rden[:],
                                            op=ALU.mult)
                    s2 = spool.tile([PT, CH], F32, tag="ln_s2")
                    nc.vector.tensor_tensor(s2[:], sv[:], sv[:],
                                            op=ALU.mult)
                    tpoly = spool.tile([PT, CH], F32, tag="ln_t")
                    nc.vector.tensor_scalar(tpoly[:], s2[:], 0.2, None,
                                            op0=ALU.mult)
                    nc.vector.scalar_tensor_tensor(
                        tpoly[:], tpoly[:], 1.0 / 3.0, s2[:],
                        op0=ALU.add, op1=ALU.mult)
                    lnm2 = spool.tile([PT, CH], F32, tag="ln_lnm2")
                    nc.vector.scalar_tensor_tensor(
                        lnm2[:], tpoly[:], 1.0, sv[:],
                        op0=ALU.add, op1=ALU.mult)
                    nc.vector.scalar_tensor_tensor(
                        lg[:], lnm2[:], 2.0, et[:],
                        op0=ALU.mult, op1=ALU.add)
                else:
                    nc.scalar.activation(lg[:], ssum[:], ACTF.Ln)
                # lse[p,t] = lg[p,t] + m[p] = lg - negm
                lse = spool.tile([PT, CH], F32, tag="lse")
                nc.vector.tensor_scalar(lse[:], lg[:], negm[:, 0:1], None,
                                        op0=ALU.subtract)
                # ll = lse + rr
                ll = spool.tile([PT, CH], F32, tag="ll")
                nc.vector.tensor_add(ll[:], lse[:], rr_all[:, ts])
                # outn = z - lse (broadcast along k)
                lse_b = lse[:].unsqueeze(2).broadcast_to([PT, CH, K])
                nc.vector.tensor_sub(outn[:, ts, :], z_all[:, ts, :], lse_b)

                nc.sync.dma_start(out=out_norm.ap()[:, ts, :],
                                  in_=outn[:, ts, :])
                nc.sync.dma_start(out=out_ll.ap()[:, ts], in_=ll[:])

            for i in range(NT):
                cols = slice(i * PT, (i + 1) * PT)

                # P = X @ Wh   -> [n, 512]
                psP = psP_pool.tile([PT, KQ], F32, tag="psP")
                nc.tensor.matmul(psP[:], xm_all[:, cols], wh_t[:],
                                 start=True, stop=True)
                # crs = X @ GC -> [n, 32]
                psC = psC_pool.tile([PT, K], F32, tag="psC")
                nc.tensor.matmul(psC[:], xa_all[:, cols], gc_t[:],
                                 start=True, stop=True)
                # rr = (X*X) @ sneg -> [n, 1]
                psR = psR_pool.tile([PT, 2], F32, tag="psR")
                nc.tensor.matmul(psR[:], x2_all[:, cols], sneg_t[:],
                                 start=True, stop=True)

                # sq = P^2 (ACT, PSUM->SBUF)
                sq = sq_pool.tile([PT, KQ], F16 if SQ_F16 else F32, tag="sq")
                nc.scalar.square(sq[:], psP[:])

                # rr slice first, on ACT (DVE is the steady-state
                # bottleneck): frees psR early
                nc.scalar.copy(rr_all[:, i:i + 1], psR[:, 0:1])

                # z0[n,k] = sum_q sq[n, k*16+q] (DVE grouped reduce; fp16
                # in+out lets the DVE run its 2x mode)
                sqg = sq[:].rearrange("p (k q) -> p k q", q=Q)
                z0 = spool.tile([PT, K], F16 if SQ_F16 else F32, tag="z0")
                with nc.allow_low_precision("z0 ~ O(30), fp16 err ~2e-2"):
                    nc.vector.tensor_reduce(z0[:], sqg, axis=AX.X, op=ALU.add)
                # z = z0 + crs (DVE reads PSUM; frees psC), then += const
                nc.vector.tensor_add(z_all[:, i, :], z0[:], psC[:])
                nc.gpsimd.tensor_add(z_all[:, i, :], z_all[:, i, :],
                                     constb_t[:])

                if (i + 1) % CH == 0:
                    phase2(i // CH)

    nc.compile()
    return nc


def host_precompute(X, log_pi, mu, Lam, log_psi):
    """Tiny O(K*D*Q) parameter factorization, in float64 for accuracy."""
    log_pi = np.asarray(log_pi, np.float64)
    mu = np.asarray(mu, np.float64)
    Lam = np.asarray(Lam, np.float64)
    log_psi = np.asarray(log_psi, np.float64)

    s = np.exp(log_psi) + 1e-5 + 1e-4                       # [D]
    sinv = 1.0 / s
    B = Lam * (s ** -0.5)[None, :, None]                    # [K,D,Q]
    M = np.eye(Q)[None] + np.einsum('kdq,kdr->kqr', B, B)   # [K,Q,Q]
    T = np.linalg.cholesky(M)
    logdet = np.sum(np.log(s)) + 2.0 * np.log(
        np.diagonal(T, axis1=1, axis2=2)).sum(1)            # [K]
    Tinv = np.linalg.inv(T)
    U = np.einsum('d,kdq,krq->kdr', sinv, Lam, Tinv)        # [K,D,Q]
    a = sinv[None, :] * mu                                  # [K,D]
    c = np.einsum('kdq,kd->kq', U, mu)                      # [K,Q]
    v = np.einsum('kdq,kq->kd', U, c)                       # [K,D]
    g = a - v                                               # [K,D]
    q1 = np.einsum('kd,kd->k', mu, a)
    q2 = np.einsum('kq,kq->k', c, c)
    const = (log_pi - 0.5 * (D * np.log(2 * np.pi) + logdet)
             - 0.5 * q1 + 0.5 * q2)                         # [K]

    # scale so that the device's grouped reduce (plain sum, or avg-pool which
    # divides by Q) yields exactly 0.5 * ||U^T x||^2
    wscale = np.sqrt(Q / 2.0) if POOL_REDUCE else np.sqrt(0.5)
    Wh = (U * wscale).transpose(0, 2, 1).reshape(KQ, D).T  # [D, KQ]
    return {
        "Wh": np.ascontiguousarray(Wh, dtype=np.float32),
        "GC": np.ascontiguousarray(g.T, dtype=np.float32),
        "sneg": np.ascontiguousarray(
            np.stack([-0.5 * sinv, np.zeros(D)], axis=1), dtype=np.float32),
        "constb": np.ascontiguousarray(
            np.broadcast_to(const[None, :], (PT, K)), dtype=np.float32),
    }


_NC_CACHE = None


def get_nc():
    global _NC_CACHE
    if _NC_CACHE is None:
        _NC_CACHE = build_bass()
    return _NC_CACHE


def kernel(X, log_pi, mu, Lam, log_psi, _collect=None):
    X = np.asarray(X, np.float32)
    params = host_precompute(X, log_pi, mu, Lam, log_psi)

    Xpad = np.zeros((NPAD, D), dtype=np.float32)
    Xpad[:N] = X
    # per-core transposed shards [D, NLOC]
    shards = Xpad.reshape(NCORES, NLOC, D)

    in_maps = [dict(params, XsT=np.ascontiguousarray(shards[c].T))
               for c in range(NCORES)]

    nc = get_nc()
    res = run_bass_kernel_spmd(nc, in_maps, list(range(NCORES)),
                               **(_collect or {}))
    if _collect is not None:
        _collect["res"] = res

    # device emits [p, t, k]; shard row n = t*128 + p
    norm = np.concatenate(
        [res.results[c]["out_norm"].transpose(1, 0, 2).reshape(NLOC, K)
         for c in range(NCORES)], axis=0)[:N]
    ll = np.concatenate(
        [res.results[c]["out_ll"].T.reshape(NLOC)
         for c in range(NCORES)], axis=0)[:N]
    return norm, ll
